# revision 5
# baseline (speedup 1.0000x reference)
"""Trainium2 Bass kernel for CompositionalResidualMLP (MoE routing, 2-node module network).

Strategy: data-parallel over batch across 8 NeuronCores. On the host, samples are
dealt round-robin (per routing pair) to cores and sorted into 64 (a0,a1) blocks
per core with per-a1 block capacities, so every layer is a dense per-module
matmul: node-0 layers see contiguous column ranges per a0 module, node-1 layers
see regular-strided column blocks per a1 module (3-level access patterns).
Activations live feature-major in SBUF ([features, samples]); matmuls are bf16.

v3 performance notes (vs v2):
- HAM duty-cycle ramp: full PE clock (2.4 GHz) is granted only after ~15us of
  sustained PE activity, and a >3.4us idle gap resets the ramp.  v2's 4
  warm-ups left a 3.4us gap before the first real matmul, so the ramp
  restarted and the first ~15us of real matmuls ran at half clock.  v3 paces
  6 warm-ups immediately after the DMA triggers (no gap into L1), moving the
  ramp into the DMA lead-in, and adds 8 tail dummies so the fixed epilogue's
  per-sem clears on the Tensor sequencer run at full clock.
- First-stream DMA triggers split across both HWDGE rings (x0c0 on SP, w00+bt
  on ACT) so the two critical transfers dispatch in parallel ~0.7us earlier.

v2 performance notes (vs v1):
- PSUM->SBUF evictions are the hard wall (Vector 0.96 GHz + Scalar 1.2 GHz are
  the only engines with a PSUM read port; 1 elem/cycle each for fp32 PSUM src).
  Evictions are load-balanced across V/S by projected cost instead of parity.
- L5 (out=32) packs 4 a1-modules into one PSUM tile via column tiling
  (tile_position auto-derived from out.base_partition), turning 16 matmuls +
  8 strided evicts + 8 strided output DMAs into 16 concurrent matmuls +
  2 flat evicts + 2 contiguous output DMAs (output returned as bf16).
- Input DMAs are coalesced (7 instead of 13) and split across the SP HWDGE
  ring (critical path: w00/x0), the ACT HWDGE ring (biases) and the gpsimd
  SWDGE ring (w1p/x1/late weights) so trigger serialization (~0.7us each)
  stops gating the first matmul.
- L2/L4 accumulation loops are k-outer so consecutive matmuls share LDWEIGHTS.
"""

import numpy as np
from contextlib import ExitStack

# Problem constants (hardcoded per contract)
B_TOT = 32768
D0 = 64
D1 = 64
M = 8          # modules per node
H = 256        # hidden width
O0 = 128
O1 = 32
NCORES = 8

WARMUP_MMS = 6     # bf16 warm-up matmuls at kernel start (HAM clock-gate warm).
                   # Enough to cover the DMA lead-in without serializing ahead
                   # of the first real matmuls in the PE FIFO; too few and the
                   # HAM re-throttles (>3.4us PE gap -> 1.2 GHz matmuls).
TAIL_MMS = 8       # dummy matmuls after L5 so the PE stays hot through the
                   # body drain: the fixed epilogue (each engine clears its
                   # ~50-sem file one EVENT_SEMAPHORE at a time) runs on the
                   # Tensor sequencer at the PE clock, and Tensor is the long
                   # pole -- at half clock it costs ~5.9us vs ~3us hot.


def _build_bass(S, Cj, off, ncol):
    """Build the per-core Bass program. All cores run the identical program on
    different data (pure SPMD, no collectives)."""
    import concourse.bass as bass
    import concourse.tile as tile
    from concourse import bacc, mybir

    F32 = mybir.dt.float32
    BF16 = mybir.dt.bfloat16
    Relu = mybir.ActivationFunctionType.Relu
    Identity = mybir.ActivationFunctionType.Identity
    Add = mybir.AluOpType.add
    Max = mybir.AluOpType.max

    DT = BF16
    half = ncol // 2
    Wblk = max(Cj)                 # uniform i-block stride inside L5 psum banks
    assert 4 * Wblk <= 512
    L5W = 8 * Wblk                 # output columns per a1-group

    nc = bacc.Bacc("TRN2", target_bir_lowering=False, debug=False,
                   enable_asserts=False, num_devices=NCORES)

    # DRAM I/O
    x0t = nc.dram_tensor("x0t", [128, half], DT, kind="ExternalInput").ap()
    x1t = nc.dram_tensor("x1t", [128, half], DT, kind="ExternalInput").ap()
    w00 = nc.dram_tensor("w00", [128, M * H // 2], DT, kind="ExternalInput").ap()
    w1p = nc.dram_tensor("w1p", [128, M * H], DT, kind="ExternalInput").ap()
    # late weights: w01 | w1a | w1o packed on the free axis
    N01 = 2 * M * O0
    NL = N01 + 3 * M * O0 + M * O1
    wlt = nc.dram_tensor("wlt", [128, NL], DT, kind="ExternalInput").ap()
    # biases: b00[16] | b01[8] | b1p[16] | b1a[8] | b1o5[2] packed, fp32
    bt = nc.dram_tensor("bt", [128, 50], F32, kind="ExternalInput").ap()
    outT = nc.dram_tensor("outT", [128, 2 * L5W], DT, kind="ExternalOutput").ap()

    def n_splits(total, cap=512):
        ns = -(-total // cap)
        base = total // ns
        rem = total - base * ns
        sizes = [base + (1 if k < rem else 0) for k in range(ns)]
        outs = []
        pos = 0
        for sz in sizes:
            outs.append((pos, sz))
            pos += sz
        return outs

    a0_splits = n_splits(S)              # contiguous column splits within a module's S columns
    assert len(a0_splits) == 2, "expect two a0 splits"
    blk_splits = n_splits(M, 4)          # i-block splits for a1-routed layers (4 blocks each)

    with tile.TileContext(nc) as tc:
        with ExitStack() as ctx:
            acts = ctx.enter_context(tc.tile_pool(name="acts", bufs=1))
            wpool = ctx.enter_context(tc.tile_pool(name="w", bufs=1))
            ps128 = ctx.enter_context(tc.tile_pool(name="ps128", bufs=4, space="PSUM"))

            # weights + biases in SBUF
            w00s = wpool.tile([128, M * H // 2], DT, tag="w00")
            w1ps = wpool.tile([128, M * H], DT, tag="w1p")
            wls = wpool.tile([128, NL], DT, tag="wl")
            w01s = wls[:, 0:2 * M * O0]
            w1as = wls[:, 2 * M * O0:5 * M * O0]
            w1os = wls[:, 5 * M * O0:5 * M * O0 + M * O1]
            bs = wpool.tile([128, 50], F32, tag="b")
            b00s = bs[:, 0:16]
            b01s = bs[:, 16:24]
            b1ps = bs[:, 24:40]
            b1as = bs[:, 40:48]
            b1o5 = bs[:, 48:50]

            x0s = acts.tile([128, half], DT, tag="x0")
            x1s = acts.tile([128, half], DT, tag="x1")
            h1a = acts.tile([128, ncol], DT, tag="h1a")
            h1b = acts.tile([128, ncol], DT, tag="h1b")
            hs = acts.tile([128, ncol], DT, tag="h")
            g1a = acts.tile([128, ncol], DT, tag="g1a")
            g1b = acts.tile([128, ncol], DT, tag="g1b")

            # ---- DMA issue: split the two critical first streams across the
            # two HWDGE rings so their triggers dispatch in parallel (~650ns
            # of queue time each): x0 chunk 0 on the SP ring, w00 + biases on
            # the ACT ring (Scalar is idle until the first eviction ~4us in).
            # The remaining streams ride the SP ring serially -- the ~650ns
            # per-trigger dispatch naturally paces them so they don't steal
            # HBM bandwidth from the critical x0/w00 transfers.
            nc.sync.dma_start(x0s[:, 0:S], x0t[:, 0:S])
            nc.scalar.dma_start(w00s[:], w00)
            nc.scalar.dma_start(bs[:], bt)
            for c in range(1, 4):
                nc.sync.dma_start(x0s[:, c * S:(c + 1) * S], x0t[:, c * S:(c + 1) * S])
            nc.sync.dma_start(w1ps[:], w1p)
            nc.sync.dma_start(x1s[:], x1t)
            nc.sync.dma_start(wls[:, 0:N01 // 2], wlt[:, 0:N01 // 2])
            nc.sync.dma_start(wls[:, N01 // 2:N01], wlt[:, N01 // 2:N01])
            nc.sync.dma_start(wls[:, N01:NL], wlt[:, N01:NL])

            # ---- HAM warm-up: continuous bf16 matmuls on scratch data from
            # right after the DMA triggers until the first real inputs land.
            # The HAM grants full PE duty (2.4 GHz) only after ~15us of
            # SUSTAINED PE activity, and a >3.4us idle gap resets the ramp --
            # so the warm-up must bridge the whole DMA lead-in with no gap,
            # making the ramp overlap the lead-in instead of the real matmuls.
            wu = wpool.tile([128, 512], BF16, tag="wu")
            nc.vector.memset(wu[:], 0.0)
            for _ in range(WARMUP_MMS):
                pw = ps128.tile([128, 1024], F32, tag="ps")
                nc.tensor.matmul(pw[:, 0:512], wu[:, 0:128], wu[:],
                                 start=True, stop=True)

            # ---- eviction engine balance: greedy assignment by projected cost
            load = [0.0, 0.0]   # ns busy: [vector, scalar]

            def evict(dst_ap, psum_ap, bias_ap, relu, fd):
                if load[0] + 125 + fd / 0.96 <= load[1] + 143 + fd / 1.2:
                    load[0] += 125 + fd / 0.96
                    if relu:
                        nc.vector.tensor_scalar(dst_ap, psum_ap, bias_ap, 0.0, Add, Max)
                    else:
                        nc.vector.tensor_scalar_add(dst_ap, psum_ap, bias_ap)
                else:
                    load[1] += 143 + fd / 1.2
                    nc.scalar.activation(dst_ap, psum_ap, Relu if relu else Identity,
                                         bias=bias_ap)

            def strided(tensor_tile, j, b0, nb, cj):
                # columns off[j] + i*S + [0, cj) for i in [b0, b0+nb)
                v = tensor_tile[:].rearrange("p (i c) -> p i c", i=M)
                return v[:, b0:b0 + nb, off[j]:off[j] + cj]

            def bank2(pt, sizes):
                v = pt[:].rearrange("p (b c) -> p b c", b=2)
                return [v[:, s, 0:sz] for s, (pos, sz) in enumerate(sizes)]

            # ---- L1: h1 = relu(W00[a0].T @ x0 + b00[a0])  [H=256 -> 2 chunks]
            # modules ip and ip+4 run concurrently on PE row-halves (K=64 each,
            # tile_position auto-derived from base partition)
            nunit = 0
            for ip in range(M // 2):
                for mo in range(2):
                    dst_tile = h1a if mo == 0 else h1b
                    wcols = slice(ip * H + mo * 128, ip * H + (mo + 1) * 128)
                    for hlf, base, bia in ((slice(0, 64), ip, mo * M + ip),
                                           (slice(64, 128), ip + 4, mo * M + ip + 4)):
                        pt = ps128.tile([128, 1024], F32, tag="ps")
                        outs2 = bank2(pt, a0_splits)
                        for s, (pos, sz) in enumerate(a0_splits):
                            nc.tensor.matmul(outs2[s], w00s[hlf, wcols],
                                             x0s[hlf, ip * S + pos: ip * S + pos + sz],
                                             start=True, stop=True)
                        sz0 = a0_splits[0][1]
                        src_ap = pt[:].rearrange("p (b c) -> p b c", b=2)[:, :, 0:sz0]
                        dst_ap = dst_tile[:, base * S: base * S + S].rearrange(
                            "p (b c) -> p b c", b=2)
                        evict(dst_ap, src_ap, b00s[:, bia: bia + 1], True, S)
                        nunit += 1

            # ---- L3: g1 = relu(W1p[a1].T @ x1 + b1p[a1])  (only needs x1)
            # i-block halves 0..3 / 4..7 run concurrently on PE row-halves
            def strided_half(tile_t, part0, j, cj):
                v = tile_t[:].rearrange("p (i c) -> p i c", i=M // 2)
                return v[part0:part0 + 64, :, off[j]:off[j] + cj]

            def l3_unit(j, mo):
                cj = Cj[j]
                dst_tile = g1a if mo == 0 else g1b
                wcols = slice(j * H + mo * 128, j * H + (mo + 1) * 128)
                pt = ps128.tile([128, 1024], F32, tag="ps")
                ptb = pt[:].rearrange("p (b c) -> p b c", b=2)
                nc.tensor.matmul(ptb[:, 0, 0:4 * cj].rearrange("p (i c) -> p i c", c=cj),
                                 w1ps[0:64, wcols],
                                 strided_half(x1s, 0, j, cj),
                                 start=True, stop=True)
                nc.tensor.matmul(ptb[:, 1, 0:4 * cj].rearrange("p (i c) -> p i c", c=cj),
                                 w1ps[64:128, wcols],
                                 strided_half(x1s, 64, j, cj),
                                 start=True, stop=True)
                src_ap = ptb[:, :, 0:4 * cj].rearrange("p b (i c) -> p b i c", c=cj)
                evict(strided(dst_tile, j, 0, 8, cj), src_ap,
                      b1ps[:, mo * M + j: mo * M + j + 1], True, 8 * cj)

            # ---- L2: h = relu(W01[a0].T @ h1 + b01[a0])  [K=256 -> 2 accum chunks]
            # k-outer so consecutive matmuls reuse the loaded weights;
            # w01 is packed module-major: chunk kc of module i at (2*i+kc)*O0
            def l2_unit(i):
                pt = ps128.tile([128, 1024], F32, tag="ps")
                outs2 = bank2(pt, a0_splits)
                for kc, src in enumerate((h1a, h1b)):
                    for s, (pos, sz) in enumerate(a0_splits):
                        nc.tensor.matmul(
                            outs2[s],
                            w01s[:, (2 * i + kc) * O0: (2 * i + kc + 1) * O0],
                            src[:, i * S + pos: i * S + pos + sz],
                            start=(kc == 0), stop=(kc == 1))
                sz0 = a0_splits[0][1]
                src_ap = pt[:].rearrange("p (b c) -> p b c", b=2)[:, :, 0:sz0]
                dst_ap = hs[:, i * S: i * S + S].rearrange("p (b c) -> p b c", b=2)
                evict(dst_ap, src_ap, b01s[:, i: i + 1], True, S)

            # Interleave L3 and L2 units: their dependency chains are
            # independent (L3: x1/w1p, L2: h1/w01), so stalls in one chain
            # are covered by ready work from the other.  L3 leads by 4 units
            # to cover w01's later DMA arrival.
            units = [("l3", j, mo) for mo in range(2) for j in range(M)]
            l2q = [("l2", i, None) for i in range(M)]
            merged = units[:4]
            rest = units[4:]
            k2 = 0
            for t, u in enumerate(rest):
                merged.append(u)
                if (t * M) // len(rest) != ((t + 1) * M) // len(rest):
                    if k2 < M:
                        merged.append(l2q[k2])
                        k2 += 1
            merged.extend(l2q[k2:])
            for kind, aa, bb in merged:
                if kind == "l3":
                    l3_unit(aa, bb)
                else:
                    l2_unit(aa)

            # ---- L4: g = relu(W1a[a1].T @ concat(h, g1) + b1a[a1]) [K=384 -> 3 chunks]
            # g reuses h1a's slot (h1 is dead after L2); k-outer for LDW reuse.
            # The g1 chunks accumulate first so L4's matmuls can start before
            # the last L2 (hs) evictions have landed.
            gs = acts.tile([128, ncol], DT, tag="h1a")
            outs_sb = acts.tile([128, 2 * L5W], DT, tag="h1b")

            def l4_unit(j):
                cj = Cj[j]
                pt = ps128.tile([128, 1024], F32, tag="ps")
                ptb = pt[:].rearrange("p (b c) -> p b c", b=2)
                for kc, src in ((1, g1a), (2, g1b), (0, hs)):
                    for s, (b0, nb) in enumerate(blk_splits):
                        ptv = ptb[:, s, 0:nb * cj].rearrange("p (i c) -> p i c", c=cj)
                        nc.tensor.matmul(
                            ptv,
                            w1as[:, (kc * M + j) * O0: (kc * M + j + 1) * O0],
                            strided(src, j, b0, nb, cj),
                            start=(kc == 1), stop=(kc == 0))
                src_ap = ptb[:, :, 0:4 * cj].rearrange("p b (i c) -> p b i c", c=cj)
                evict(strided(gs, j, 0, 8, cj), src_ap,
                      b1as[:, j: j + 1], True, 8 * cj)

            # ---- L5: out = W1o[a1].T @ g + b1o[a1]  (identity, out=32)
            # 4 a1-modules packed per PSUM tile via column tiling: module j of
            # group G lands on psum partitions 32*(j-4G)..+32.  i-blocks are
            # written at a uniform stride Wblk so one flat evict covers the
            # whole group; the output DMA is then a contiguous [128, L5W] blob.
            # Each group is emitted right after its 4 L4 units so the L5 tail
            # overlaps the other group's L4 work.
            def l5_group(G):
                pt = ps128.tile([128, 1024], F32, tag="ps")
                for jb in range(4):
                    j = 4 * G + jb
                    cj = Cj[j]
                    band = pt[32 * jb:32 * jb + 32, :]
                    for s, (b0, nb) in enumerate(blk_splits):
                        ptv = band.rearrange("p (b r) -> p b r", b=2)[:, s, 0:nb * Wblk]
                        ptv = ptv.rearrange("p (i c) -> p i c", c=Wblk)[:, :, 0:cj]
                        nc.tensor.matmul(
                            ptv,
                            w1os[:, j * O1: (j + 1) * O1],
                            strided(gs, j, b0, nb, cj),
                            start=True, stop=True,
                            tile_position=(0, 32 * jb))
                # split the eviction across both engines (one bank each) so
                # the final-group latency is halved on the critical tail
                for s in range(2):
                    src_ap = pt[:, s * 512:s * 512 + 4 * Wblk]
                    dst_ap = outs_sb[:, G * L5W + s * 4 * Wblk:
                                     G * L5W + (s + 1) * 4 * Wblk]
                    if s == 0:
                        nc.vector.tensor_scalar_add(dst_ap, src_ap, b1o5[:, G:G + 1])
                    else:
                        nc.scalar.activation(dst_ap, src_ap, Identity,
                                             bias=b1o5[:, G:G + 1])
                nc.sync.dma_start(outT[:, G * L5W:(G + 1) * L5W],
                                  outs_sb[:, G * L5W:(G + 1) * L5W])

            for j in range(4):
                l4_unit(j)
            l5_group(0)
            for j in range(4, M):
                l4_unit(j)
            l5_group(1)

            # ---- tail dummies: keep the PE duty-cycle grant alive while the
            # final evictions / output DMA drain, so the epilogue's Tensor
            # sem-clears run at full sequencer clock.
            for _ in range(TAIL_MMS):
                pw = ps128.tile([128, 1024], F32, tag="ps")
                nc.tensor.matmul(pw[:, 0:512], wu[:, 0:128], wu[:],
                                 start=True, stop=True)

    nc.compile()
    return nc


def _pack_weights(inputs):
    """Pack per-module weight stacks into SBUF-image layouts (contraction dim on
    partitions, [K<=128, chunks*modules*out] on the free axis)."""
    import ml_dtypes
    wdt = ml_dtypes.bfloat16
    f = lambda a: np.ascontiguousarray(a.astype(wdt))
    W00 = inputs["W00"]; W01 = inputs["W01"]; W1p = inputs["W1p"]
    W1a = inputs["W1a"]; W1o = inputs["W1o"]
    w00lo = W00[:M // 2].transpose(1, 0, 2).reshape(D0, M * H // 2)
    w00hi = W00[M // 2:].transpose(1, 0, 2).reshape(D0, M * H // 2)
    w1pp = W1p.transpose(1, 0, 2).reshape(D1, M * H)
    w01 = W01.reshape(M, 2, 128, O0).transpose(2, 0, 1, 3).reshape(128, 2 * M * O0)
    w1a = W1a.reshape(M, 3, 128, O0).transpose(2, 1, 0, 3).reshape(128, 3 * M * O0)
    w1o = W1o.transpose(1, 0, 2).reshape(128, M * O1)
    # biases: b00[16] | b01[8] | b1p[16] | b1a[8] | b1o5[2], fp32 [128, 50]
    b00 = inputs["b00"].reshape(M, 2, 128).transpose(2, 1, 0).reshape(128, 2 * M)
    b1p = inputs["b1p"].reshape(M, 2, 128).transpose(2, 1, 0).reshape(128, 2 * M)
    b01 = np.broadcast_to(inputs["b01"].T, (128, M))
    b1a = np.broadcast_to(inputs["b1a"].T, (128, M))
    # L5 col-tiled bias: partition 32*jb + ch of group G = b1o[4G + jb, ch]
    b1o5 = inputs["b1o"].reshape(2, 4 * O1).T               # [128, 2]
    biases = np.concatenate([b00, b01, b1p, b1a, b1o5], axis=1).astype(np.float32)
    return {
        "w00": f(np.concatenate([w00lo, w00hi], axis=0)),
        "w1p": f(np.concatenate([w1pp, w1pp], axis=0)),
        "wlt": f(np.concatenate([w01, w1a, w1o], axis=1)),
        "bt": np.ascontiguousarray(biases),
    }


def _route(input_val):
    """Assign each sample to a (core, column) in the blocked layout."""
    a0 = np.argmax(input_val[:, D0 + D1: D0 + D1 + M], axis=1)
    a1 = np.argmax(input_val[:, D0 + D1 + M: D0 + D1 + 2 * M], axis=1)
    B = input_val.shape[0]
    nij = np.zeros((M, M), dtype=np.int64)
    np.add.at(nij, (a0, a1), 1)
    Cj = np.maximum(-(-nij.max(axis=0) // NCORES), 64)
    off = np.concatenate([[0], np.cumsum(Cj)[:-1]]).astype(np.int64)
    S = int(Cj.sum())
    ncol = M * S

    pairkey = a0 * M + a1
    order = np.argsort(pairkey, kind="stable")
    counts = np.bincount(pairkey, minlength=M * M)
    group_start = np.concatenate([[0], np.cumsum(counts)[:-1]])
    rank_sorted = np.arange(B) - np.repeat(group_start, counts)
    rank = np.empty(B, dtype=np.int64)
    rank[order] = rank_sorted
    core = rank % NCORES
    slot = rank // NCORES
    assert np.all(slot < Cj[a1]), "capacity overflow"
    col = a0 * S + off[a1] + slot
    return core, col, a0, a1, slot, S, [int(c) for c in Cj], [int(o) for o in off], ncol


def kernel(**inputs):
    import os
    import ml_dtypes
    from concourse.bass_utils import run_bass_kernel_spmd

    input_val = np.asarray(inputs["input_val"], dtype=np.float32)
    B = input_val.shape[0]

    core, col, a0, a1, slot, S, Cj, off, ncol = _route(input_val)

    xdt = ml_dtypes.bfloat16
    feat0 = input_val[:, :D0]
    feat1 = input_val[:, D0:D0 + D1]
    # split layout: module blocks i<4 (by a0) on partitions 0-63, i>=4 on 64-127
    half = ncol // 2
    X0T = np.zeros((NCORES, 128, half), dtype=xdt)
    X1T = np.zeros((NCORES, 128, half), dtype=xdt)
    hi = col >= half
    prow = np.where(hi, 64, 0)
    pcol = np.where(hi, col - half, col)
    for r in (0, 64):
        m = prow == r
        X0T[core[m], r:r + 64, pcol[m]] = feat0[m].astype(xdt)
        X1T[core[m], r:r + 64, pcol[m]] = feat1[m].astype(xdt)

    wmap = _pack_weights({k: np.asarray(v, dtype=np.float32) for k, v in inputs.items()
                          if k != "input_val"})

    nc = _build_bass(S, Cj, off, ncol)

    in_maps = [dict(wmap, x0t=np.ascontiguousarray(X0T[c]),
                    x1t=np.ascontiguousarray(X1T[c])) for c in range(NCORES)]
    res = run_bass_kernel_spmd(nc, in_maps, core_ids=list(range(NCORES)),
                               tmpdir=os.environ.get("BASS_TMPDIR"))
    global _LAST_RESULTS
    _LAST_RESULTS = res

    # unshard: OUT[core, 32*(a1%4)+ch, (a1>=4)*L5W + (a0>=4)*4*Wblk + (a0%4)*Wblk + slot]
    Wblk = max(Cj)
    L5W = 8 * Wblk
    OUT = np.stack([np.asarray(r["outT"], dtype=np.float32) for r in res.results])
    OUT = OUT.reshape(NCORES, 4, O1, 2 * L5W)
    colnew = (a1 // 4) * L5W + (a0 // 4) * 4 * Wblk + (a0 % 4) * Wblk + slot
    out = OUT[core, a1 % 4, :, colnew]
    return np.ascontiguousarray(out).astype(np.float32)



# revision 7
# speedup vs baseline: 1.1783x; 1.1783x over previous
"""Trainium2 Bass kernel for CompositionalResidualMLP (MoE routing, 2-node module network).

Strategy: data-parallel over batch across 8 NeuronCores. On the host, samples are
dealt round-robin (per routing pair) to cores and sorted into 64 (a0,a1) blocks
per core with per-a1 block capacities, so every layer is a dense per-module
matmul: node-0 layers see contiguous column ranges per a0 module, node-1 layers
see regular-strided column blocks per a1 module (3-level access patterns).
Activations live feature-major in SBUF ([features, samples]); matmuls are bf16.

v3 performance notes (vs v2):
- HAM duty-cycle ramp: full PE clock (2.4 GHz) is granted only after ~15us of
  sustained PE activity, and a >3.4us idle gap resets the ramp.  v2's 4
  warm-ups left a 3.4us gap before the first real matmul, so the ramp
  restarted and the first ~15us of real matmuls ran at half clock.  v3 paces
  6 warm-ups immediately after the DMA triggers (no gap into L1), moving the
  ramp into the DMA lead-in, and adds 8 tail dummies so the fixed epilogue's
  per-sem clears on the Tensor sequencer run at full clock.
- First-stream DMA triggers split across both HWDGE rings (x0c0 on SP, w00+bt
  on ACT) so the two critical transfers dispatch in parallel ~0.7us earlier.

v2 performance notes (vs v1):
- PSUM->SBUF evictions are the hard wall (Vector 0.96 GHz + Scalar 1.2 GHz are
  the only engines with a PSUM read port; 1 elem/cycle each for fp32 PSUM src).
  Evictions are load-balanced across V/S by projected cost instead of parity.
- L5 (out=32) packs 4 a1-modules into one PSUM tile via column tiling
  (tile_position auto-derived from out.base_partition), turning 16 matmuls +
  8 strided evicts + 8 strided output DMAs into 16 concurrent matmuls +
  2 flat evicts + 2 contiguous output DMAs (output returned as bf16).
- Input DMAs are coalesced (7 instead of 13) and split across the SP HWDGE
  ring (critical path: w00/x0), the ACT HWDGE ring (biases) and the gpsimd
  SWDGE ring (w1p/x1/late weights) so trigger serialization (~0.7us each)
  stops gating the first matmul.
- L2/L4 accumulation loops are k-outer so consecutive matmuls share LDWEIGHTS.
"""

import numpy as np
from contextlib import ExitStack

# Problem constants (hardcoded per contract)
B_TOT = 32768
D0 = 64
D1 = 64
M = 8          # modules per node
H = 256        # hidden width
O0 = 128
O1 = 32
NCORES = 8

WARMUP_MMS = 2     # bf16 warm-up matmuls at kernel start.  Measurement note:
                   # the full-duty grant (matmul issue 129ns vs 253ns per
                   # ~300-col matmul) lands ~27us after NEFF start on most
                   # cores regardless of early PE activity, so warm-ups do
                   # NOT accelerate it -- and dense extra PE work appears to
                   # lower the granted rate (chip-level power budget).  Keep
                   # only a tiny pipeline warm-up.
TAIL_MMS = 0       # tail dummies extended the matmul era 1:1 and bought
                   # nothing on the epilogue -- keep disabled.


def _build_bass(S, Cj, off, ncol):
    """Build the per-core Bass program. All cores run the identical program on
    different data (pure SPMD, no collectives)."""
    import concourse.bass as bass
    import concourse.tile as tile
    from concourse import bacc, mybir

    F32 = mybir.dt.float32
    BF16 = mybir.dt.bfloat16
    Relu = mybir.ActivationFunctionType.Relu
    Identity = mybir.ActivationFunctionType.Identity
    Add = mybir.AluOpType.add
    Max = mybir.AluOpType.max

    DT = BF16
    half = ncol // 2
    Wblk = max(Cj)                 # uniform i-block stride inside L5 psum banks
    assert 4 * Wblk <= 512
    L5W = 8 * Wblk                 # output columns per a1-group

    nc = bacc.Bacc("TRN2", target_bir_lowering=False, debug=False,
                   enable_asserts=False, num_devices=NCORES)

    # DRAM I/O
    x0t = nc.dram_tensor("x0t", [128, half], DT, kind="ExternalInput").ap()
    x1t = nc.dram_tensor("x1t", [128, half], DT, kind="ExternalInput").ap()
    w00 = nc.dram_tensor("w00", [128, M * H // 2], DT, kind="ExternalInput").ap()
    w1p = nc.dram_tensor("w1p", [128, M * H], DT, kind="ExternalInput").ap()
    # late weights: w01 | w1a | w1o packed on the free axis
    N01 = 2 * M * O0
    NL = N01 + 3 * M * O0 + M * O1
    wlt = nc.dram_tensor("wlt", [128, NL], DT, kind="ExternalInput").ap()
    # biases: b00[16] | b01[8] | b1p[16] | b1a[8] | b1o5[2] packed, fp32
    bt = nc.dram_tensor("bt", [128, 50], F32, kind="ExternalInput").ap()
    outT = nc.dram_tensor("outT", [128, 2 * L5W], DT, kind="ExternalOutput").ap()

    def n_splits(total, cap=512):
        ns = -(-total // cap)
        base = total // ns
        rem = total - base * ns
        sizes = [base + (1 if k < rem else 0) for k in range(ns)]
        outs = []
        pos = 0
        for sz in sizes:
            outs.append((pos, sz))
            pos += sz
        return outs

    a0_splits = n_splits(S)              # contiguous column splits within a module's S columns
    assert len(a0_splits) == 2, "expect two a0 splits"
    blk_splits = n_splits(M, 4)          # i-block splits for a1-routed layers (4 blocks each)

    with tile.TileContext(nc) as tc:
        with ExitStack() as ctx:
            acts = ctx.enter_context(tc.tile_pool(name="acts", bufs=1))
            wpool = ctx.enter_context(tc.tile_pool(name="w", bufs=1))
            ps128 = ctx.enter_context(tc.tile_pool(name="ps128", bufs=4, space="PSUM"))

            # weights + biases in SBUF
            w00s = wpool.tile([128, M * H // 2], DT, tag="w00")
            w1ps = wpool.tile([128, M * H], DT, tag="w1p")
            wls = wpool.tile([128, NL], DT, tag="wl")
            w01s = wls[:, 0:2 * M * O0]
            w1as = wls[:, 2 * M * O0:5 * M * O0]
            w1os = wls[:, 5 * M * O0:5 * M * O0 + M * O1]
            bs = wpool.tile([128, 50], F32, tag="b")
            b00s = bs[:, 0:16]
            b01s = bs[:, 16:24]
            b1ps = bs[:, 24:40]
            b1as = bs[:, 40:48]
            b1o5 = bs[:, 48:50]

            x0s = acts.tile([128, half], DT, tag="x0")
            x1s = acts.tile([128, half], DT, tag="x1")
            h1a = acts.tile([128, ncol], DT, tag="h1a")
            h1b = acts.tile([128, ncol], DT, tag="h1b")
            hs = acts.tile([128, ncol], DT, tag="h")
            g1a = acts.tile([128, ncol], DT, tag="g1a")
            g1b = acts.tile([128, ncol], DT, tag="g1b")

            # ---- DMA issue: split the two critical first streams across the
            # two HWDGE rings so their triggers dispatch in parallel (~650ns
            # of queue time each): x0 chunk 0 on the SP ring, w00 + biases on
            # the ACT ring (Scalar is idle until the first eviction ~4us in).
            # The remaining streams ride the SP ring serially -- the ~650ns
            # per-trigger dispatch naturally paces them so they don't steal
            # HBM bandwidth from the critical x0/w00 transfers.
            nc.sync.dma_start(x0s[:, 0:S], x0t[:, 0:S])
            nc.scalar.dma_start(w00s[:], w00)
            nc.scalar.dma_start(bs[:], bt)
            for c in range(1, 4):
                nc.sync.dma_start(x0s[:, c * S:(c + 1) * S], x0t[:, c * S:(c + 1) * S])
            nc.sync.dma_start(w1ps[:], w1p)
            nc.sync.dma_start(x1s[:], x1t)
            # late weights on the ACT ring: Scalar's trigger queue is free
            # until the first eviction, so w01 dispatches by ~9us and lands
            # before the L2 LDWEIGHTS (the SP-ring sem-row-reuse gating made
            # it ~1.5us late there, stalling L2 units ~0.5us each).
            nc.scalar.dma_start(wls[:, 0:N01 // 2], wlt[:, 0:N01 // 2])
            nc.scalar.dma_start(wls[:, N01 // 2:N01], wlt[:, N01 // 2:N01])
            nc.sync.dma_start(wls[:, N01:NL], wlt[:, N01:NL])

            # ---- HAM warm-up: continuous bf16 matmuls on scratch data from
            # right after the DMA triggers until the first real inputs land.
            # The HAM grants full PE duty (2.4 GHz) only after ~15us of
            # SUSTAINED PE activity, and a >3.4us idle gap resets the ramp --
            # so the warm-up must bridge the whole DMA lead-in with no gap,
            # making the ramp overlap the lead-in instead of the real matmuls.
            wu = wpool.tile([128, 512], BF16, tag="wu")
            nc.vector.memset(wu[:], 0.0)
            for _ in range(WARMUP_MMS):
                pw = ps128.tile([128, 1024], F32, tag="ps")
                nc.tensor.matmul(pw[:, 0:512], wu[:, 0:128], wu[:],
                                 start=True, stop=True)

            # ---- eviction engine balance: greedy assignment by projected cost
            load = [0.0, 0.0]   # ns busy: [vector, scalar]

            def evict(dst_ap, psum_ap, bias_ap, relu, fd):
                if load[0] + 125 + fd / 0.96 <= load[1] + 143 + fd / 1.2:
                    load[0] += 125 + fd / 0.96
                    if relu:
                        nc.vector.tensor_scalar(dst_ap, psum_ap, bias_ap, 0.0, Add, Max)
                    else:
                        nc.vector.tensor_scalar_add(dst_ap, psum_ap, bias_ap)
                else:
                    load[1] += 143 + fd / 1.2
                    nc.scalar.activation(dst_ap, psum_ap, Relu if relu else Identity,
                                         bias=bias_ap)

            def strided(tensor_tile, j, b0, nb, cj):
                # columns off[j] + i*S + [0, cj) for i in [b0, b0+nb)
                v = tensor_tile[:].rearrange("p (i c) -> p i c", i=M)
                return v[:, b0:b0 + nb, off[j]:off[j] + cj]

            def bank2(pt, sizes):
                v = pt[:].rearrange("p (b c) -> p b c", b=2)
                return [v[:, s, 0:sz] for s, (pos, sz) in enumerate(sizes)]

            # ---- L1: h1 = relu(W00[a0].T @ x0 + b00[a0])  [H=256 -> 2 chunks]
            # modules ip and ip+4 run concurrently on PE row-halves (K=64 each,
            # tile_position auto-derived from base partition)
            nunit = 0
            for ip in range(M // 2):
                for mo in range(2):
                    dst_tile = h1a if mo == 0 else h1b
                    wcols = slice(ip * H + mo * 128, ip * H + (mo + 1) * 128)
                    for hlf, base, bia in ((slice(0, 64), ip, mo * M + ip),
                                           (slice(64, 128), ip + 4, mo * M + ip + 4)):
                        pt = ps128.tile([128, 1024], F32, tag="ps")
                        outs2 = bank2(pt, a0_splits)
                        for s, (pos, sz) in enumerate(a0_splits):
                            nc.tensor.matmul(outs2[s], w00s[hlf, wcols],
                                             x0s[hlf, ip * S + pos: ip * S + pos + sz],
                                             start=True, stop=True)
                        sz0 = a0_splits[0][1]
                        src_ap = pt[:].rearrange("p (b c) -> p b c", b=2)[:, :, 0:sz0]
                        dst_ap = dst_tile[:, base * S: base * S + S].rearrange(
                            "p (b c) -> p b c", b=2)
                        evict(dst_ap, src_ap, b00s[:, bia: bia + 1], True, S)
                        nunit += 1

            # ---- L3: g1 = relu(W1p[a1].T @ x1 + b1p[a1])  (only needs x1)
            # i-block halves 0..3 / 4..7 run concurrently on PE row-halves
            def strided_half(tile_t, part0, j, cj):
                v = tile_t[:].rearrange("p (i c) -> p i c", i=M // 2)
                return v[part0:part0 + 64, :, off[j]:off[j] + cj]

            def l3_unit(j, mo):
                cj = Cj[j]
                dst_tile = g1a if mo == 0 else g1b
                wcols = slice(j * H + mo * 128, j * H + (mo + 1) * 128)
                pt = ps128.tile([128, 1024], F32, tag="ps")
                ptb = pt[:].rearrange("p (b c) -> p b c", b=2)
                nc.tensor.matmul(ptb[:, 0, 0:4 * cj].rearrange("p (i c) -> p i c", c=cj),
                                 w1ps[0:64, wcols],
                                 strided_half(x1s, 0, j, cj),
                                 start=True, stop=True)
                nc.tensor.matmul(ptb[:, 1, 0:4 * cj].rearrange("p (i c) -> p i c", c=cj),
                                 w1ps[64:128, wcols],
                                 strided_half(x1s, 64, j, cj),
                                 start=True, stop=True)
                src_ap = ptb[:, :, 0:4 * cj].rearrange("p b (i c) -> p b i c", c=cj)
                evict(strided(dst_tile, j, 0, 8, cj), src_ap,
                      b1ps[:, mo * M + j: mo * M + j + 1], True, 8 * cj)

            # ---- L2: h = relu(W01[a0].T @ h1 + b01[a0])  [K=256 -> 2 accum chunks]
            # k-outer so consecutive matmuls reuse the loaded weights;
            # w01 is packed module-major: chunk kc of module i at (2*i+kc)*O0
            def l2_unit(i):
                pt = ps128.tile([128, 1024], F32, tag="ps")
                outs2 = bank2(pt, a0_splits)
                for kc, src in enumerate((h1a, h1b)):
                    for s, (pos, sz) in enumerate(a0_splits):
                        nc.tensor.matmul(
                            outs2[s],
                            w01s[:, (2 * i + kc) * O0: (2 * i + kc + 1) * O0],
                            src[:, i * S + pos: i * S + pos + sz],
                            start=(kc == 0), stop=(kc == 1))
                sz0 = a0_splits[0][1]
                src_ap = pt[:].rearrange("p (b c) -> p b c", b=2)[:, :, 0:sz0]
                dst_ap = hs[:, i * S: i * S + S].rearrange("p (b c) -> p b c", b=2)
                evict(dst_ap, src_ap, b01s[:, i: i + 1], True, S)

            # Interleave L3 and L2 units: their dependency chains are
            # independent (L3: x1/w1p, L2: h1/w01), so stalls in one chain
            # are covered by ready work from the other.  L3 leads by 4 units
            # to cover w01's later DMA arrival.
            units = [("l3", j, mo) for mo in range(2) for j in range(M)]
            l2q = [("l2", i, None) for i in range(M)]
            merged = units[:4]
            rest = units[4:]
            k2 = 0
            for t, u in enumerate(rest):
                merged.append(u)
                if (t * M) // len(rest) != ((t + 1) * M) // len(rest):
                    if k2 < M:
                        merged.append(l2q[k2])
                        k2 += 1
            merged.extend(l2q[k2:])
            for kind, aa, bb in merged:
                if kind == "l3":
                    l3_unit(aa, bb)
                else:
                    l2_unit(aa)

            # ---- L4: g = relu(W1a[a1].T @ concat(h, g1) + b1a[a1]) [K=384 -> 3 chunks]
            # g reuses h1a's slot (h1 is dead after L2); k-outer for LDW reuse.
            # The g1 chunks accumulate first so L4's matmuls can start before
            # the last L2 (hs) evictions have landed.
            gs = acts.tile([128, ncol], DT, tag="h1a")
            outs_sb = acts.tile([128, 2 * L5W], DT, tag="h1b")

            def l4_unit(j):
                cj = Cj[j]
                pt = ps128.tile([128, 1024], F32, tag="ps")
                ptb = pt[:].rearrange("p (b c) -> p b c", b=2)
                for kc, src in ((1, g1a), (2, g1b), (0, hs)):
                    for s, (b0, nb) in enumerate(blk_splits):
                        ptv = ptb[:, s, 0:nb * cj].rearrange("p (i c) -> p i c", c=cj)
                        nc.tensor.matmul(
                            ptv,
                            w1as[:, (kc * M + j) * O0: (kc * M + j + 1) * O0],
                            strided(src, j, b0, nb, cj),
                            start=(kc == 1), stop=(kc == 0))
                src_ap = ptb[:, :, 0:4 * cj].rearrange("p b (i c) -> p b i c", c=cj)
                evict(strided(gs, j, 0, 8, cj), src_ap,
                      b1as[:, j: j + 1], True, 8 * cj)

            # ---- L5: out = W1o[a1].T @ g + b1o[a1]  (identity, out=32)
            # 4 a1-modules packed per PSUM tile via column tiling: module j of
            # group G lands on psum partitions 32*(j-4G)..+32.  i-blocks are
            # written at a uniform stride Wblk so one flat evict covers the
            # whole group; the output DMA is then a contiguous [128, L5W] blob.
            # Each group is emitted right after its 4 L4 units so the L5 tail
            # overlaps the other group's L4 work.
            def l5_group(G):
                pt = ps128.tile([128, 1024], F32, tag="ps")
                for jb in range(4):
                    j = 4 * G + jb
                    cj = Cj[j]
                    band = pt[32 * jb:32 * jb + 32, :]
                    for s, (b0, nb) in enumerate(blk_splits):
                        ptv = band.rearrange("p (b r) -> p b r", b=2)[:, s, 0:nb * Wblk]
                        ptv = ptv.rearrange("p (i c) -> p i c", c=Wblk)[:, :, 0:cj]
                        nc.tensor.matmul(
                            ptv,
                            w1os[:, j * O1: (j + 1) * O1],
                            strided(gs, j, b0, nb, cj),
                            start=True, stop=True,
                            tile_position=(0, 32 * jb))
                # split the eviction across both engines (one bank each) so
                # the final-group latency is halved on the critical tail
                for s in range(2):
                    src_ap = pt[:, s * 512:s * 512 + 4 * Wblk]
                    dst_ap = outs_sb[:, G * L5W + s * 4 * Wblk:
                                     G * L5W + (s + 1) * 4 * Wblk]
                    if s == 0:
                        nc.vector.tensor_scalar_add(dst_ap, src_ap, b1o5[:, G:G + 1])
                    else:
                        nc.scalar.activation(dst_ap, src_ap, Identity,
                                             bias=b1o5[:, G:G + 1])
                nc.sync.dma_start(outT[:, G * L5W:(G + 1) * L5W],
                                  outs_sb[:, G * L5W:(G + 1) * L5W])

            for j in range(4):
                l4_unit(j)
            l5_group(0)
            for j in range(4, M):
                l4_unit(j)
            l5_group(1)

            # ---- tail dummies: keep the PE duty-cycle grant alive while the
            # final evictions / output DMA drain, so the epilogue's Tensor
            # sem-clears run at full sequencer clock.
            for _ in range(TAIL_MMS):
                pw = ps128.tile([128, 1024], F32, tag="ps")
                nc.tensor.matmul(pw[:, 0:512], wu[:, 0:128], wu[:],
                                 start=True, stop=True)

    nc.compile()
    return nc


def _pack_weights(inputs):
    """Pack per-module weight stacks into SBUF-image layouts (contraction dim on
    partitions, [K<=128, chunks*modules*out] on the free axis)."""
    import ml_dtypes
    wdt = ml_dtypes.bfloat16
    f = lambda a: np.ascontiguousarray(a.astype(wdt))
    W00 = inputs["W00"]; W01 = inputs["W01"]; W1p = inputs["W1p"]
    W1a = inputs["W1a"]; W1o = inputs["W1o"]
    w00lo = W00[:M // 2].transpose(1, 0, 2).reshape(D0, M * H // 2)
    w00hi = W00[M // 2:].transpose(1, 0, 2).reshape(D0, M * H // 2)
    w1pp = W1p.transpose(1, 0, 2).reshape(D1, M * H)
    w01 = W01.reshape(M, 2, 128, O0).transpose(2, 0, 1, 3).reshape(128, 2 * M * O0)
    w1a = W1a.reshape(M, 3, 128, O0).transpose(2, 1, 0, 3).reshape(128, 3 * M * O0)
    w1o = W1o.transpose(1, 0, 2).reshape(128, M * O1)
    # biases: b00[16] | b01[8] | b1p[16] | b1a[8] | b1o5[2], fp32 [128, 50]
    b00 = inputs["b00"].reshape(M, 2, 128).transpose(2, 1, 0).reshape(128, 2 * M)
    b1p = inputs["b1p"].reshape(M, 2, 128).transpose(2, 1, 0).reshape(128, 2 * M)
    b01 = np.broadcast_to(inputs["b01"].T, (128, M))
    b1a = np.broadcast_to(inputs["b1a"].T, (128, M))
    # L5 col-tiled bias: partition 32*jb + ch of group G = b1o[4G + jb, ch]
    b1o5 = inputs["b1o"].reshape(2, 4 * O1).T               # [128, 2]
    biases = np.concatenate([b00, b01, b1p, b1a, b1o5], axis=1).astype(np.float32)
    return {
        "w00": f(np.concatenate([w00lo, w00hi], axis=0)),
        "w1p": f(np.concatenate([w1pp, w1pp], axis=0)),
        "wlt": f(np.concatenate([w01, w1a, w1o], axis=1)),
        "bt": np.ascontiguousarray(biases),
    }


def _route(input_val):
    """Assign each sample to a (core, column) in the blocked layout."""
    a0 = np.argmax(input_val[:, D0 + D1: D0 + D1 + M], axis=1)
    a1 = np.argmax(input_val[:, D0 + D1 + M: D0 + D1 + 2 * M], axis=1)
    B = input_val.shape[0]
    nij = np.zeros((M, M), dtype=np.int64)
    np.add.at(nij, (a0, a1), 1)
    Cj = np.maximum(-(-nij.max(axis=0) // NCORES), 64)
    off = np.concatenate([[0], np.cumsum(Cj)[:-1]]).astype(np.int64)
    S = int(Cj.sum())
    ncol = M * S

    pairkey = a0 * M + a1
    order = np.argsort(pairkey, kind="stable")
    counts = np.bincount(pairkey, minlength=M * M)
    group_start = np.concatenate([[0], np.cumsum(counts)[:-1]])
    rank_sorted = np.arange(B) - np.repeat(group_start, counts)
    rank = np.empty(B, dtype=np.int64)
    rank[order] = rank_sorted
    core = rank % NCORES
    slot = rank // NCORES
    assert np.all(slot < Cj[a1]), "capacity overflow"
    col = a0 * S + off[a1] + slot
    return core, col, a0, a1, slot, S, [int(c) for c in Cj], [int(o) for o in off], ncol


def kernel(**inputs):
    import os
    import ml_dtypes
    from concourse.bass_utils import run_bass_kernel_spmd

    input_val = np.asarray(inputs["input_val"], dtype=np.float32)
    B = input_val.shape[0]

    core, col, a0, a1, slot, S, Cj, off, ncol = _route(input_val)

    xdt = ml_dtypes.bfloat16
    feat0 = input_val[:, :D0]
    feat1 = input_val[:, D0:D0 + D1]
    # split layout: module blocks i<4 (by a0) on partitions 0-63, i>=4 on 64-127
    half = ncol // 2
    X0T = np.zeros((NCORES, 128, half), dtype=xdt)
    X1T = np.zeros((NCORES, 128, half), dtype=xdt)
    hi = col >= half
    prow = np.where(hi, 64, 0)
    pcol = np.where(hi, col - half, col)
    for r in (0, 64):
        m = prow == r
        X0T[core[m], r:r + 64, pcol[m]] = feat0[m].astype(xdt)
        X1T[core[m], r:r + 64, pcol[m]] = feat1[m].astype(xdt)

    wmap = _pack_weights({k: np.asarray(v, dtype=np.float32) for k, v in inputs.items()
                          if k != "input_val"})

    nc = _build_bass(S, Cj, off, ncol)

    in_maps = [dict(wmap, x0t=np.ascontiguousarray(X0T[c]),
                    x1t=np.ascontiguousarray(X1T[c])) for c in range(NCORES)]
    res = run_bass_kernel_spmd(nc, in_maps, core_ids=list(range(NCORES)),
                               tmpdir=os.environ.get("BASS_TMPDIR"))
    global _LAST_RESULTS
    _LAST_RESULTS = res

    # unshard: OUT[core, 32*(a1%4)+ch, (a1>=4)*L5W + (a0>=4)*4*Wblk + (a0%4)*Wblk + slot]
    Wblk = max(Cj)
    L5W = 8 * Wblk
    OUT = np.stack([np.asarray(r["outT"], dtype=np.float32) for r in res.results])
    OUT = OUT.reshape(NCORES, 4, O1, 2 * L5W)
    colnew = (a1 // 4) * L5W + (a0 // 4) * 4 * Wblk + (a0 % 4) * Wblk + slot
    out = OUT[core, a1 % 4, :, colnew]
    return np.ascontiguousarray(out).astype(np.float32)



# revision 11
# speedup vs baseline: 1.1947x; 1.0139x over previous
"""Trainium2 Bass kernel for CompositionalResidualMLP (MoE routing, 2-node module network).

Strategy: data-parallel over batch across 8 NeuronCores. On the host, samples are
dealt round-robin (per routing pair) to cores and sorted into 64 (a0,a1) blocks
per core with per-a1 block capacities, so every layer is a dense per-module
matmul: node-0 layers see contiguous column ranges per a0 module, node-1 layers
see regular-strided column blocks per a1 module (3-level access patterns).
Activations live feature-major in SBUF ([features, samples]); matmuls are bf16.

v3 performance notes (vs v2):
- HAM duty-cycle ramp: full PE clock (2.4 GHz) is granted only after ~15us of
  sustained PE activity, and a >3.4us idle gap resets the ramp.  v2's 4
  warm-ups left a 3.4us gap before the first real matmul, so the ramp
  restarted and the first ~15us of real matmuls ran at half clock.  v3 paces
  6 warm-ups immediately after the DMA triggers (no gap into L1), moving the
  ramp into the DMA lead-in, and adds 8 tail dummies so the fixed epilogue's
  per-sem clears on the Tensor sequencer run at full clock.
- First-stream DMA triggers split across both HWDGE rings (x0c0 on SP, w00+bt
  on ACT) so the two critical transfers dispatch in parallel ~0.7us earlier.

v2 performance notes (vs v1):
- PSUM->SBUF evictions are the hard wall (Vector 0.96 GHz + Scalar 1.2 GHz are
  the only engines with a PSUM read port; 1 elem/cycle each for fp32 PSUM src).
  Evictions are load-balanced across V/S by projected cost instead of parity.
- L5 (out=32) packs 4 a1-modules into one PSUM tile via column tiling
  (tile_position auto-derived from out.base_partition), turning 16 matmuls +
  8 strided evicts + 8 strided output DMAs into 16 concurrent matmuls +
  2 flat evicts + 2 contiguous output DMAs (output returned as bf16).
- Input DMAs are coalesced (7 instead of 13) and split across the SP HWDGE
  ring (critical path: w00/x0), the ACT HWDGE ring (biases) and the gpsimd
  SWDGE ring (w1p/x1/late weights) so trigger serialization (~0.7us each)
  stops gating the first matmul.
- L2/L4 accumulation loops are k-outer so consecutive matmuls share LDWEIGHTS.
"""

import numpy as np
from contextlib import ExitStack

# Problem constants (hardcoded per contract)
B_TOT = 32768
D0 = 64
D1 = 64
M = 8          # modules per node
H = 256        # hidden width
O0 = 128
O1 = 32
NCORES = 8

WARMUP_MMS = 2     # bf16 warm-up matmuls at kernel start.  Measurement note:
                   # the full-duty grant (matmul issue 129ns vs 253ns per
                   # ~300-col matmul) lands ~27us after NEFF start on most
                   # cores regardless of early PE activity, so warm-ups do
                   # NOT accelerate it -- and dense extra PE work appears to
                   # lower the granted rate (chip-level power budget).  Keep
                   # only a tiny pipeline warm-up.
TAIL_MMS = 16      # gated tail dummies (see emission site): run in the
                   # PE-idle shadow of the output-DMA drain to keep the
                   # duty-cycle grant hot into the fixed sem-clear epilogue.


def _build_bass(S, Cj, off, ncol):
    """Build the per-core Bass program. All cores run the identical program on
    different data (pure SPMD, no collectives)."""
    import concourse.bass as bass
    import concourse.tile as tile
    from concourse import bacc, mybir

    F32 = mybir.dt.float32
    BF16 = mybir.dt.bfloat16
    Relu = mybir.ActivationFunctionType.Relu
    Identity = mybir.ActivationFunctionType.Identity
    Add = mybir.AluOpType.add
    Max = mybir.AluOpType.max

    DT = BF16
    half = ncol // 2
    Wblk = max(Cj)                 # uniform i-block stride inside L5 psum banks
    assert 4 * Wblk <= 512
    L5W = 8 * Wblk                 # output columns per a1-group

    nc = bacc.Bacc("TRN2", target_bir_lowering=False, debug=False,
                   enable_asserts=False, num_devices=NCORES)

    # DRAM I/O
    x0t = nc.dram_tensor("x0t", [128, half], DT, kind="ExternalInput").ap()
    x1t = nc.dram_tensor("x1t", [128, half], DT, kind="ExternalInput").ap()
    w00 = nc.dram_tensor("w00", [128, M * H // 2], DT, kind="ExternalInput").ap()
    w1p = nc.dram_tensor("w1p", [128, M * H], DT, kind="ExternalInput").ap()
    # late weights: w01 | w1a | w1o packed on the free axis
    N01 = 2 * M * O0
    NL = N01 + 3 * M * O0 + M * O1
    wlt = nc.dram_tensor("wlt", [128, NL], DT, kind="ExternalInput").ap()
    # biases: b00[16] | b01[8] | b1p[16] | b1a[8] | b1o5[2] packed, fp32
    bt = nc.dram_tensor("bt", [128, 50], F32, kind="ExternalInput").ap()
    outT = nc.dram_tensor("outT", [128, 2 * L5W], DT, kind="ExternalOutput").ap()

    def n_splits(total, cap=512):
        ns = -(-total // cap)
        base = total // ns
        rem = total - base * ns
        sizes = [base + (1 if k < rem else 0) for k in range(ns)]
        outs = []
        pos = 0
        for sz in sizes:
            outs.append((pos, sz))
            pos += sz
        return outs

    a0_splits = n_splits(S)              # contiguous column splits within a module's S columns
    assert len(a0_splits) == 2, "expect two a0 splits"
    blk_splits = n_splits(M, 4)          # i-block splits for a1-routed layers (4 blocks each)

    with tile.TileContext(nc) as tc:
        with ExitStack() as ctx:
            acts = ctx.enter_context(tc.tile_pool(name="acts", bufs=1))
            wpool = ctx.enter_context(tc.tile_pool(name="w", bufs=1))
            ps128 = ctx.enter_context(tc.tile_pool(name="ps128", bufs=4, space="PSUM"))

            # weights + biases in SBUF
            w00s = wpool.tile([128, M * H // 2], DT, tag="w00")
            w1ps = wpool.tile([128, M * H], DT, tag="w1p")
            wls = wpool.tile([128, NL], DT, tag="wl")
            w01s = wls[:, 0:2 * M * O0]
            w1as = wls[:, 2 * M * O0:5 * M * O0]
            w1os = wls[:, 5 * M * O0:5 * M * O0 + M * O1]
            bs = wpool.tile([128, 50], F32, tag="b")
            b00s = bs[:, 0:16]
            b01s = bs[:, 16:24]
            b1ps = bs[:, 24:40]
            b1as = bs[:, 40:48]
            b1o5 = bs[:, 48:50]

            x0s = acts.tile([128, half], DT, tag="x0")
            x1s = acts.tile([128, half], DT, tag="x1")
            h1a = acts.tile([128, ncol], DT, tag="h1a")
            h1b = acts.tile([128, ncol], DT, tag="h1b")
            hs = acts.tile([128, ncol], DT, tag="h")
            g1a = acts.tile([128, ncol], DT, tag="g1a")
            g1b = acts.tile([128, ncol], DT, tag="g1b")

            # ---- DMA issue: split the two critical first streams across the
            # two HWDGE rings so their triggers dispatch in parallel (~650ns
            # of queue time each): x0 chunk 0 on the SP ring, w00 + biases on
            # the ACT ring (Scalar is idle until the first eviction ~4us in).
            # The remaining streams ride the SP ring serially -- the ~650ns
            # per-trigger dispatch naturally paces them so they don't steal
            # HBM bandwidth from the critical x0/w00 transfers.
            # All input streams ride the SP ring, serially paced by the
            # ~650ns per-trigger dispatch, ordered by when the consumer
            # needs them (w00/x0 for L1 first, then w1p/x1 for L3, then the
            # late weights).  Two rings streaming concurrently just split
            # the HBM bandwidth round-robin and make the critical stream
            # late (measured: x1 at 20.4us when wlt rode the ACT ring).
            # Only the tiny bias tensor rides the ACT ring.
            nc.sync.dma_start(w00s[:], w00)
            nc.scalar.dma_start(bs[:], bt)
            nc.sync.dma_start(x0s[:, 0:S], x0t[:, 0:S])
            nc.sync.dma_start(x0s[:, S:2 * S], x0t[:, S:2 * S])
            nc.sync.dma_start(x0s[:, 2 * S:3 * S], x0t[:, 2 * S:3 * S])
            nc.sync.dma_start(w1ps[:], w1p)
            nc.sync.dma_start(x0s[:, 3 * S:4 * S], x0t[:, 3 * S:4 * S])
            nc.sync.dma_start(x1s[:], x1t)
            nc.sync.dma_start(wls[:, 0:N01 // 2], wlt[:, 0:N01 // 2])
            nc.sync.dma_start(wls[:, N01 // 2:N01], wlt[:, N01 // 2:N01])
            nc.sync.dma_start(wls[:, N01:NL], wlt[:, N01:NL])

            # ---- HAM warm-up: continuous bf16 matmuls on scratch data from
            # right after the DMA triggers until the first real inputs land.
            # The HAM grants full PE duty (2.4 GHz) only after ~15us of
            # SUSTAINED PE activity, and a >3.4us idle gap resets the ramp --
            # so the warm-up must bridge the whole DMA lead-in with no gap,
            # making the ramp overlap the lead-in instead of the real matmuls.
            wu = wpool.tile([128, 512], BF16, tag="wu")
            nc.vector.memset(wu[:], 0.0)
            for _ in range(WARMUP_MMS):
                pw = ps128.tile([128, 1024], F32, tag="ps")
                nc.tensor.matmul(pw[:, 0:512], wu[:, 0:128], wu[:],
                                 start=True, stop=True)

            # ---- eviction engine balance: greedy assignment by projected cost
            load = [0.0, 0.0]   # ns busy: [vector, scalar]

            def evict(dst_ap, psum_ap, bias_ap, relu, fd):
                if load[0] + 125 + fd / 0.96 <= load[1] + 143 + fd / 1.2:
                    load[0] += 125 + fd / 0.96
                    if relu:
                        nc.vector.tensor_scalar(dst_ap, psum_ap, bias_ap, 0.0, Add, Max)
                    else:
                        nc.vector.tensor_scalar_add(dst_ap, psum_ap, bias_ap)
                else:
                    load[1] += 143 + fd / 1.2
                    nc.scalar.activation(dst_ap, psum_ap, Relu if relu else Identity,
                                         bias=bias_ap)

            def strided(tensor_tile, j, b0, nb, cj):
                # columns off[j] + i*S + [0, cj) for i in [b0, b0+nb)
                v = tensor_tile[:].rearrange("p (i c) -> p i c", i=M)
                return v[:, b0:b0 + nb, off[j]:off[j] + cj]

            def bank2(pt, sizes):
                v = pt[:].rearrange("p (b c) -> p b c", b=2)
                return [v[:, s, 0:sz] for s, (pos, sz) in enumerate(sizes)]

            # ---- L1: h1 = relu(W00[a0].T @ x0 + b00[a0])  [H=256 -> 2 chunks]
            # modules ip and ip+4 run concurrently on PE row-halves (K=64 each,
            # tile_position auto-derived from base partition)
            nunit = 0
            for ip in range(M // 2):
                for mo in range(2):
                    dst_tile = h1a if mo == 0 else h1b
                    wcols = slice(ip * H + mo * 128, ip * H + (mo + 1) * 128)
                    for hlf, base, bia in ((slice(0, 64), ip, mo * M + ip),
                                           (slice(64, 128), ip + 4, mo * M + ip + 4)):
                        pt = ps128.tile([128, 1024], F32, tag="ps")
                        outs2 = bank2(pt, a0_splits)
                        for s, (pos, sz) in enumerate(a0_splits):
                            nc.tensor.matmul(outs2[s], w00s[hlf, wcols],
                                             x0s[hlf, ip * S + pos: ip * S + pos + sz],
                                             start=True, stop=True)
                        sz0 = a0_splits[0][1]
                        src_ap = pt[:].rearrange("p (b c) -> p b c", b=2)[:, :, 0:sz0]
                        dst_ap = dst_tile[:, base * S: base * S + S].rearrange(
                            "p (b c) -> p b c", b=2)
                        evict(dst_ap, src_ap, b00s[:, bia: bia + 1], True, S)
                        nunit += 1

            # ---- L3: g1 = relu(W1p[a1].T @ x1 + b1p[a1])  (only needs x1)
            # i-block halves 0..3 / 4..7 run concurrently on PE row-halves
            def strided_half(tile_t, part0, j, cj):
                v = tile_t[:].rearrange("p (i c) -> p i c", i=M // 2)
                return v[part0:part0 + 64, :, off[j]:off[j] + cj]

            def l3_unit(j, mo):
                cj = Cj[j]
                dst_tile = g1a if mo == 0 else g1b
                wcols = slice(j * H + mo * 128, j * H + (mo + 1) * 128)
                pt = ps128.tile([128, 1024], F32, tag="ps")
                ptb = pt[:].rearrange("p (b c) -> p b c", b=2)
                nc.tensor.matmul(ptb[:, 0, 0:4 * cj].rearrange("p (i c) -> p i c", c=cj),
                                 w1ps[0:64, wcols],
                                 strided_half(x1s, 0, j, cj),
                                 start=True, stop=True)
                nc.tensor.matmul(ptb[:, 1, 0:4 * cj].rearrange("p (i c) -> p i c", c=cj),
                                 w1ps[64:128, wcols],
                                 strided_half(x1s, 64, j, cj),
                                 start=True, stop=True)
                src_ap = ptb[:, :, 0:4 * cj].rearrange("p b (i c) -> p b i c", c=cj)
                evict(strided(dst_tile, j, 0, 8, cj), src_ap,
                      b1ps[:, mo * M + j: mo * M + j + 1], True, 8 * cj)

            # ---- L2: h = relu(W01[a0].T @ h1 + b01[a0])  [K=256 -> 2 accum chunks]
            # k-outer so consecutive matmuls reuse the loaded weights;
            # w01 is packed module-major: chunk kc of module i at (2*i+kc)*O0
            def l2_unit(i):
                pt = ps128.tile([128, 1024], F32, tag="ps")
                outs2 = bank2(pt, a0_splits)
                for kc, src in enumerate((h1a, h1b)):
                    for s, (pos, sz) in enumerate(a0_splits):
                        nc.tensor.matmul(
                            outs2[s],
                            w01s[:, (2 * i + kc) * O0: (2 * i + kc + 1) * O0],
                            src[:, i * S + pos: i * S + pos + sz],
                            start=(kc == 0), stop=(kc == 1))
                sz0 = a0_splits[0][1]
                src_ap = pt[:].rearrange("p (b c) -> p b c", b=2)[:, :, 0:sz0]
                dst_ap = hs[:, i * S: i * S + S].rearrange("p (b c) -> p b c", b=2)
                evict(dst_ap, src_ap, b01s[:, i: i + 1], True, S)

            # Interleave L3 and L2 units: their dependency chains are
            # independent (L3: x1/w1p, L2: h1/w01), so stalls in one chain
            # are covered by ready work from the other.  L3 leads by 4 units
            # to cover w01's later DMA arrival.
            units = [("l3", j, mo) for mo in range(2) for j in range(M)]
            l2q = [("l2", i, None) for i in range(M)]
            merged = units[:4]
            rest = units[4:]
            k2 = 0
            for t, u in enumerate(rest):
                merged.append(u)
                if (t * M) // len(rest) != ((t + 1) * M) // len(rest):
                    if k2 < M:
                        merged.append(l2q[k2])
                        k2 += 1
            merged.extend(l2q[k2:])
            for kind, aa, bb in merged:
                if kind == "l3":
                    l3_unit(aa, bb)
                else:
                    l2_unit(aa)

            # ---- L4: g = relu(W1a[a1].T @ concat(h, g1) + b1a[a1]) [K=384 -> 3 chunks]
            # g reuses h1a's slot (h1 is dead after L2); k-outer for LDW reuse.
            # The g1 chunks accumulate first so L4's matmuls can start before
            # the last L2 (hs) evictions have landed.
            gs = acts.tile([128, ncol], DT, tag="h1a")
            outs_sb = acts.tile([128, 2 * L5W], DT, tag="h1b")

            def l4_unit(j):
                cj = Cj[j]
                pt = ps128.tile([128, 1024], F32, tag="ps")
                ptb = pt[:].rearrange("p (b c) -> p b c", b=2)
                for kc, src in ((1, g1a), (2, g1b), (0, hs)):
                    for s, (b0, nb) in enumerate(blk_splits):
                        ptv = ptb[:, s, 0:nb * cj].rearrange("p (i c) -> p i c", c=cj)
                        nc.tensor.matmul(
                            ptv,
                            w1as[:, (kc * M + j) * O0: (kc * M + j + 1) * O0],
                            strided(src, j, b0, nb, cj),
                            start=(kc == 1), stop=(kc == 0))
                src_ap = ptb[:, :, 0:4 * cj].rearrange("p b (i c) -> p b i c", c=cj)
                evict(strided(gs, j, 0, 8, cj), src_ap,
                      b1as[:, j: j + 1], True, 8 * cj)

            # ---- L5: out = W1o[a1].T @ g + b1o[a1]  (identity, out=32)
            # 4 a1-modules packed per PSUM tile via column tiling: module j of
            # group G lands on psum partitions 32*(j-4G)..+32.  i-blocks are
            # written at a uniform stride Wblk so one flat evict covers the
            # whole group; the output DMA is then a contiguous [128, L5W] blob.
            # Each group is emitted right after its 4 L4 units so the L5 tail
            # overlaps the other group's L4 work.
            def l5_group(G):
                pt = ps128.tile([128, 1024], F32, tag="ps")
                for jb in range(4):
                    j = 4 * G + jb
                    cj = Cj[j]
                    band = pt[32 * jb:32 * jb + 32, :]
                    for s, (b0, nb) in enumerate(blk_splits):
                        ptv = band.rearrange("p (b r) -> p b r", b=2)[:, s, 0:nb * Wblk]
                        ptv = ptv.rearrange("p (i c) -> p i c", c=Wblk)[:, :, 0:cj]
                        nc.tensor.matmul(
                            ptv,
                            w1os[:, j * O1: (j + 1) * O1],
                            strided(gs, j, b0, nb, cj),
                            start=True, stop=True,
                            tile_position=(0, 32 * jb))
                # split the eviction across both engines (one bank each) so
                # the final-group latency is halved on the critical tail
                # per-bank evict + per-bank output DMA so the first half's
                # transfer starts while the second half is still evicting
                for s in range(2):
                    src_ap = pt[:, s * 512:s * 512 + 4 * Wblk]
                    dst_ap = outs_sb[:, G * L5W + s * 4 * Wblk:
                                     G * L5W + (s + 1) * 4 * Wblk]
                    if s == 0:
                        nc.vector.tensor_scalar_add(dst_ap, src_ap, b1o5[:, G:G + 1])
                    else:
                        nc.scalar.activation(dst_ap, src_ap, Identity,
                                             bias=b1o5[:, G:G + 1])
                    nc.sync.dma_start(
                        outT[:, G * L5W + s * 4 * Wblk:G * L5W + (s + 1) * 4 * Wblk],
                        outs_sb[:, G * L5W + s * 4 * Wblk:G * L5W + (s + 1) * 4 * Wblk])

            for j in range(4):
                l4_unit(j)
            l5_group(0)
            for j in range(4, M):
                l4_unit(j)
            l5_group(1)

            # ---- gated tail dummies: the first one READS the final L5
            # eviction's SBUF output, so it fires only once the last real
            # work lands; the rest chain after it.  They run entirely in the
            # PE-idle shadow of the output-DMA drain (no body extension) and
            # keep the full duty-cycle grant alive into the epilogue, where
            # the Tensor sequencer's ~50 per-sem clears run at the PE clock
            # (57ns hot vs 115ns throttled).
            if TAIL_MMS:
                last = outs_sb[:, 2 * L5W - 128:2 * L5W]
                pw = ps128.tile([128, 1024], F32, tag="ps")
                nc.tensor.matmul(pw[:, 0:128], wu[:, 0:128], last,
                                 start=True, stop=True)
                for _ in range(TAIL_MMS - 1):
                    pw = ps128.tile([128, 1024], F32, tag="ps")
                    nc.tensor.matmul(pw[:, 0:128], wu[:, 0:128], last,
                                     start=True, stop=True)

    nc.compile()
    return nc


def _pack_weights(inputs):
    """Pack per-module weight stacks into SBUF-image layouts (contraction dim on
    partitions, [K<=128, chunks*modules*out] on the free axis)."""
    import ml_dtypes
    wdt = ml_dtypes.bfloat16
    f = lambda a: np.ascontiguousarray(a.astype(wdt))
    W00 = inputs["W00"]; W01 = inputs["W01"]; W1p = inputs["W1p"]
    W1a = inputs["W1a"]; W1o = inputs["W1o"]
    w00lo = W00[:M // 2].transpose(1, 0, 2).reshape(D0, M * H // 2)
    w00hi = W00[M // 2:].transpose(1, 0, 2).reshape(D0, M * H // 2)
    w1pp = W1p.transpose(1, 0, 2).reshape(D1, M * H)
    w01 = W01.reshape(M, 2, 128, O0).transpose(2, 0, 1, 3).reshape(128, 2 * M * O0)
    w1a = W1a.reshape(M, 3, 128, O0).transpose(2, 1, 0, 3).reshape(128, 3 * M * O0)
    w1o = W1o.transpose(1, 0, 2).reshape(128, M * O1)
    # biases: b00[16] | b01[8] | b1p[16] | b1a[8] | b1o5[2], fp32 [128, 50]
    b00 = inputs["b00"].reshape(M, 2, 128).transpose(2, 1, 0).reshape(128, 2 * M)
    b1p = inputs["b1p"].reshape(M, 2, 128).transpose(2, 1, 0).reshape(128, 2 * M)
    b01 = np.broadcast_to(inputs["b01"].T, (128, M))
    b1a = np.broadcast_to(inputs["b1a"].T, (128, M))
    # L5 col-tiled bias: partition 32*jb + ch of group G = b1o[4G + jb, ch]
    b1o5 = inputs["b1o"].reshape(2, 4 * O1).T               # [128, 2]
    biases = np.concatenate([b00, b01, b1p, b1a, b1o5], axis=1).astype(np.float32)
    return {
        "w00": f(np.concatenate([w00lo, w00hi], axis=0)),
        "w1p": f(np.concatenate([w1pp, w1pp], axis=0)),
        "wlt": f(np.concatenate([w01, w1a, w1o], axis=1)),
        "bt": np.ascontiguousarray(biases),
    }


def _route(input_val):
    """Assign each sample to a (core, column) in the blocked layout."""
    a0 = np.argmax(input_val[:, D0 + D1: D0 + D1 + M], axis=1)
    a1 = np.argmax(input_val[:, D0 + D1 + M: D0 + D1 + 2 * M], axis=1)
    B = input_val.shape[0]
    nij = np.zeros((M, M), dtype=np.int64)
    np.add.at(nij, (a0, a1), 1)
    Cj = np.maximum(-(-nij.max(axis=0) // NCORES), 64)
    off = np.concatenate([[0], np.cumsum(Cj)[:-1]]).astype(np.int64)
    S = int(Cj.sum())
    ncol = M * S

    pairkey = a0 * M + a1
    order = np.argsort(pairkey, kind="stable")
    counts = np.bincount(pairkey, minlength=M * M)
    group_start = np.concatenate([[0], np.cumsum(counts)[:-1]])
    rank_sorted = np.arange(B) - np.repeat(group_start, counts)
    rank = np.empty(B, dtype=np.int64)
    rank[order] = rank_sorted
    core = rank % NCORES
    slot = rank // NCORES
    assert np.all(slot < Cj[a1]), "capacity overflow"
    col = a0 * S + off[a1] + slot
    return core, col, a0, a1, slot, S, [int(c) for c in Cj], [int(o) for o in off], ncol


def kernel(**inputs):
    import os
    import ml_dtypes
    from concourse.bass_utils import run_bass_kernel_spmd

    input_val = np.asarray(inputs["input_val"], dtype=np.float32)
    B = input_val.shape[0]

    core, col, a0, a1, slot, S, Cj, off, ncol = _route(input_val)

    xdt = ml_dtypes.bfloat16
    feat0 = input_val[:, :D0]
    feat1 = input_val[:, D0:D0 + D1]
    # split layout: module blocks i<4 (by a0) on partitions 0-63, i>=4 on 64-127
    half = ncol // 2
    X0T = np.zeros((NCORES, 128, half), dtype=xdt)
    X1T = np.zeros((NCORES, 128, half), dtype=xdt)
    hi = col >= half
    prow = np.where(hi, 64, 0)
    pcol = np.where(hi, col - half, col)
    for r in (0, 64):
        m = prow == r
        X0T[core[m], r:r + 64, pcol[m]] = feat0[m].astype(xdt)
        X1T[core[m], r:r + 64, pcol[m]] = feat1[m].astype(xdt)

    wmap = _pack_weights({k: np.asarray(v, dtype=np.float32) for k, v in inputs.items()
                          if k != "input_val"})

    nc = _build_bass(S, Cj, off, ncol)

    in_maps = [dict(wmap, x0t=np.ascontiguousarray(X0T[c]),
                    x1t=np.ascontiguousarray(X1T[c])) for c in range(NCORES)]
    res = run_bass_kernel_spmd(nc, in_maps, core_ids=list(range(NCORES)),
                               tmpdir=os.environ.get("BASS_TMPDIR"))
    global _LAST_RESULTS
    _LAST_RESULTS = res

    # unshard: OUT[core, 32*(a1%4)+ch, (a1>=4)*L5W + (a0>=4)*4*Wblk + (a0%4)*Wblk + slot]
    Wblk = max(Cj)
    L5W = 8 * Wblk
    OUT = np.stack([np.asarray(r["outT"], dtype=np.float32) for r in res.results])
    OUT = OUT.reshape(NCORES, 4, O1, 2 * L5W)
    colnew = (a1 // 4) * L5W + (a0 // 4) * 4 * Wblk + (a0 % 4) * Wblk + slot
    out = OUT[core, a1 % 4, :, colnew]
    return np.ascontiguousarray(out).astype(np.float32)



# revision 13
# speedup vs baseline: 1.1985x; 1.0032x over previous
"""Trainium2 Bass kernel for CompositionalResidualMLP (MoE routing, 2-node module network).

Strategy: data-parallel over batch across 8 NeuronCores. On the host, samples are
dealt round-robin (per routing pair) to cores and sorted into 64 (a0,a1) blocks
per core with per-a1 block capacities, so every layer is a dense per-module
matmul: node-0 layers see contiguous column ranges per a0 module, node-1 layers
see regular-strided column blocks per a1 module (3-level access patterns).
Activations live feature-major in SBUF ([features, samples]); matmuls are bf16.

v3 performance notes (vs v2):
- HAM duty-cycle ramp: full PE clock (2.4 GHz) is granted only after ~15us of
  sustained PE activity, and a >3.4us idle gap resets the ramp.  v2's 4
  warm-ups left a 3.4us gap before the first real matmul, so the ramp
  restarted and the first ~15us of real matmuls ran at half clock.  v3 paces
  6 warm-ups immediately after the DMA triggers (no gap into L1), moving the
  ramp into the DMA lead-in, and adds 8 tail dummies so the fixed epilogue's
  per-sem clears on the Tensor sequencer run at full clock.
- First-stream DMA triggers split across both HWDGE rings (x0c0 on SP, w00+bt
  on ACT) so the two critical transfers dispatch in parallel ~0.7us earlier.

v2 performance notes (vs v1):
- PSUM->SBUF evictions are the hard wall (Vector 0.96 GHz + Scalar 1.2 GHz are
  the only engines with a PSUM read port; 1 elem/cycle each for fp32 PSUM src).
  Evictions are load-balanced across V/S by projected cost instead of parity.
- L5 (out=32) packs 4 a1-modules into one PSUM tile via column tiling
  (tile_position auto-derived from out.base_partition), turning 16 matmuls +
  8 strided evicts + 8 strided output DMAs into 16 concurrent matmuls +
  2 flat evicts + 2 contiguous output DMAs (output returned as bf16).
- Input DMAs are coalesced (7 instead of 13) and split across the SP HWDGE
  ring (critical path: w00/x0), the ACT HWDGE ring (biases) and the gpsimd
  SWDGE ring (w1p/x1/late weights) so trigger serialization (~0.7us each)
  stops gating the first matmul.
- L2/L4 accumulation loops are k-outer so consecutive matmuls share LDWEIGHTS.
"""

import numpy as np
from contextlib import ExitStack

# Problem constants (hardcoded per contract)
B_TOT = 32768
D0 = 64
D1 = 64
M = 8          # modules per node
H = 256        # hidden width
O0 = 128
O1 = 32
NCORES = 8

WARMUP_MMS = 2     # bf16 warm-up matmuls at kernel start.  Measurement note:
                   # the full-duty grant (matmul issue 129ns vs 253ns per
                   # ~300-col matmul) lands ~27us after NEFF start on most
                   # cores regardless of early PE activity, so warm-ups do
                   # NOT accelerate it -- and dense extra PE work appears to
                   # lower the granted rate (chip-level power budget).  Keep
                   # only a tiny pipeline warm-up.
TAIL_MMS = 0       # tail dummies: measured useless -- the full-duty window
                   # lasts a fixed ~17us from its grant regardless of PE
                   # activity, so the epilogue sem-clears always run at the
                   # throttled sequencer rate (115ns each).


def _build_bass(S, Cj, off, ncol):
    """Build the per-core Bass program. All cores run the identical program on
    different data (pure SPMD, no collectives)."""
    import concourse.bass as bass
    import concourse.tile as tile
    from concourse import bacc, mybir

    F32 = mybir.dt.float32
    BF16 = mybir.dt.bfloat16
    Relu = mybir.ActivationFunctionType.Relu
    Identity = mybir.ActivationFunctionType.Identity
    Add = mybir.AluOpType.add
    Max = mybir.AluOpType.max

    DT = BF16
    half = ncol // 2
    Wblk = max(Cj)                 # uniform i-block stride inside L5 psum banks
    assert 4 * Wblk <= 512
    L5W = 8 * Wblk                 # output columns per a1-group

    nc = bacc.Bacc("TRN2", target_bir_lowering=False, debug=False,
                   enable_asserts=False, num_devices=NCORES)

    # DRAM I/O
    x0t = nc.dram_tensor("x0t", [128, half], DT, kind="ExternalInput").ap()
    x1t = nc.dram_tensor("x1t", [128, half], DT, kind="ExternalInput").ap()
    w00 = nc.dram_tensor("w00", [128, M * H // 2], DT, kind="ExternalInput").ap()
    w1p = nc.dram_tensor("w1p", [128, M * H], DT, kind="ExternalInput").ap()
    # late weights: w01 | w1a | w1o packed on the free axis
    N01 = 2 * M * O0
    NL = N01 + 3 * M * O0 + M * O1
    wlt = nc.dram_tensor("wlt", [128, NL], DT, kind="ExternalInput").ap()
    # biases: b00[16] | b01[8] | b1p[16] | b1a[8] | b1o5[2] packed, fp32
    bt = nc.dram_tensor("bt", [128, 50], F32, kind="ExternalInput").ap()
    outT = nc.dram_tensor("outT", [128, 2 * L5W], DT, kind="ExternalOutput").ap()

    def n_splits(total, cap=512):
        ns = -(-total // cap)
        base = total // ns
        rem = total - base * ns
        sizes = [base + (1 if k < rem else 0) for k in range(ns)]
        outs = []
        pos = 0
        for sz in sizes:
            outs.append((pos, sz))
            pos += sz
        return outs

    a0_splits = n_splits(S)              # contiguous column splits within a module's S columns
    assert len(a0_splits) == 2, "expect two a0 splits"
    blk_splits = n_splits(M, 4)          # i-block splits for a1-routed layers (4 blocks each)

    with tile.TileContext(nc) as tc:
        with ExitStack() as ctx:
            acts = ctx.enter_context(tc.tile_pool(name="acts", bufs=1))
            wpool = ctx.enter_context(tc.tile_pool(name="w", bufs=1))
            ps128 = ctx.enter_context(tc.tile_pool(name="ps128", bufs=4, space="PSUM"))

            # weights + biases in SBUF
            w00s = wpool.tile([128, M * H // 2], DT, tag="w00")
            w1ps = wpool.tile([128, M * H], DT, tag="w1p")
            wls = wpool.tile([128, NL], DT, tag="wl")
            w01s = wls[:, 0:2 * M * O0]
            w1as = wls[:, 2 * M * O0:5 * M * O0]
            w1os = wls[:, 5 * M * O0:5 * M * O0 + M * O1]
            bs = wpool.tile([128, 50], F32, tag="b")
            b00s = bs[:, 0:16]
            b01s = bs[:, 16:24]
            b1ps = bs[:, 24:40]
            b1as = bs[:, 40:48]
            b1o5 = bs[:, 48:50]

            x0s = acts.tile([128, half], DT, tag="x0")
            x1s = acts.tile([128, half], DT, tag="x1")
            h1a = acts.tile([128, ncol], DT, tag="h1a")
            h1b = acts.tile([128, ncol], DT, tag="h1b")
            hs = acts.tile([128, ncol], DT, tag="h")
            g1a = acts.tile([128, ncol], DT, tag="g1a")
            g1b = acts.tile([128, ncol], DT, tag="g1b")

            # ---- DMA issue: split the two critical first streams across the
            # two HWDGE rings so their triggers dispatch in parallel (~650ns
            # of queue time each): x0 chunk 0 on the SP ring, w00 + biases on
            # the ACT ring (Scalar is idle until the first eviction ~4us in).
            # The remaining streams ride the SP ring serially -- the ~650ns
            # per-trigger dispatch naturally paces them so they don't steal
            # HBM bandwidth from the critical x0/w00 transfers.
            # All input streams ride the SP ring, serially paced by the
            # ~650ns per-trigger dispatch, ordered by when the consumer
            # needs them (w00/x0 for L1 first, then w1p/x1 for L3, then the
            # late weights).  Two rings streaming concurrently just split
            # the HBM bandwidth round-robin and make the critical stream
            # late (measured: x1 at 20.4us when wlt rode the ACT ring).
            # Only the tiny bias tensor rides the ACT ring.
            nc.sync.dma_start(w00s[:, 0:M * H // 4], w00[:, 0:M * H // 4])
            nc.scalar.dma_start(bs[:], bt)
            nc.sync.dma_start(x0s[:, 0:S], x0t[:, 0:S])
            nc.sync.dma_start(w00s[:, M * H // 4:], w00[:, M * H // 4:])
            nc.sync.dma_start(x0s[:, S:2 * S], x0t[:, S:2 * S])
            nc.sync.dma_start(x0s[:, 2 * S:3 * S], x0t[:, 2 * S:3 * S])
            nc.sync.dma_start(x0s[:, 3 * S:4 * S], x0t[:, 3 * S:4 * S])
            nc.sync.dma_start(wls[:, 0:N01 // 2], wlt[:, 0:N01 // 2])
            nc.sync.dma_start(w1ps[:], w1p)
            nc.sync.dma_start(x1s[:], x1t)
            nc.sync.dma_start(wls[:, N01 // 2:N01], wlt[:, N01 // 2:N01])
            nc.sync.dma_start(wls[:, N01:NL], wlt[:, N01:NL])

            # ---- HAM warm-up: continuous bf16 matmuls on scratch data from
            # right after the DMA triggers until the first real inputs land.
            # The HAM grants full PE duty (2.4 GHz) only after ~15us of
            # SUSTAINED PE activity, and a >3.4us idle gap resets the ramp --
            # so the warm-up must bridge the whole DMA lead-in with no gap,
            # making the ramp overlap the lead-in instead of the real matmuls.
            wu = wpool.tile([128, 512], BF16, tag="wu")
            nc.vector.memset(wu[:], 0.0)
            for _ in range(WARMUP_MMS):
                pw = ps128.tile([128, 1024], F32, tag="ps")
                nc.tensor.matmul(pw[:, 0:512], wu[:, 0:128], wu[:],
                                 start=True, stop=True)

            # ---- eviction engine balance: greedy assignment by projected cost
            load = [0.0, 0.0]   # ns busy: [vector, scalar]

            def evict(dst_ap, psum_ap, bias_ap, relu, fd):
                if load[0] + 125 + fd / 0.96 <= load[1] + 143 + fd / 1.2:
                    load[0] += 125 + fd / 0.96
                    if relu:
                        nc.vector.tensor_scalar(dst_ap, psum_ap, bias_ap, 0.0, Add, Max)
                    else:
                        nc.vector.tensor_scalar_add(dst_ap, psum_ap, bias_ap)
                else:
                    load[1] += 143 + fd / 1.2
                    nc.scalar.activation(dst_ap, psum_ap, Relu if relu else Identity,
                                         bias=bias_ap)

            def strided(tensor_tile, j, b0, nb, cj):
                # columns off[j] + i*S + [0, cj) for i in [b0, b0+nb)
                v = tensor_tile[:].rearrange("p (i c) -> p i c", i=M)
                return v[:, b0:b0 + nb, off[j]:off[j] + cj]

            def bank2(pt, sizes):
                v = pt[:].rearrange("p (b c) -> p b c", b=2)
                return [v[:, s, 0:sz] for s, (pos, sz) in enumerate(sizes)]

            # ---- L1: h1 = relu(W00[a0].T @ x0 + b00[a0])  [H=256 -> 2 chunks]
            # modules ip and ip+4 run concurrently on PE row-halves (K=64 each,
            # tile_position auto-derived from base partition)
            nunit = 0
            for ip in range(M // 2):
                for mo in range(2):
                    dst_tile = h1a if mo == 0 else h1b
                    wcols = slice(ip * H + mo * 128, ip * H + (mo + 1) * 128)
                    for hlf, base, bia in ((slice(0, 64), ip, mo * M + ip),
                                           (slice(64, 128), ip + 4, mo * M + ip + 4)):
                        pt = ps128.tile([128, 1024], F32, tag="ps")
                        outs2 = bank2(pt, a0_splits)
                        for s, (pos, sz) in enumerate(a0_splits):
                            nc.tensor.matmul(outs2[s], w00s[hlf, wcols],
                                             x0s[hlf, ip * S + pos: ip * S + pos + sz],
                                             start=True, stop=True)
                        sz0 = a0_splits[0][1]
                        src_ap = pt[:].rearrange("p (b c) -> p b c", b=2)[:, :, 0:sz0]
                        dst_ap = dst_tile[:, base * S: base * S + S].rearrange(
                            "p (b c) -> p b c", b=2)
                        evict(dst_ap, src_ap, b00s[:, bia: bia + 1], True, S)
                        nunit += 1

            # ---- L3: g1 = relu(W1p[a1].T @ x1 + b1p[a1])  (only needs x1)
            # i-block halves 0..3 / 4..7 run concurrently on PE row-halves
            def strided_half(tile_t, part0, j, cj):
                v = tile_t[:].rearrange("p (i c) -> p i c", i=M // 2)
                return v[part0:part0 + 64, :, off[j]:off[j] + cj]

            def l3_unit(j, mo):
                cj = Cj[j]
                dst_tile = g1a if mo == 0 else g1b
                wcols = slice(j * H + mo * 128, j * H + (mo + 1) * 128)
                pt = ps128.tile([128, 1024], F32, tag="ps")
                ptb = pt[:].rearrange("p (b c) -> p b c", b=2)
                nc.tensor.matmul(ptb[:, 0, 0:4 * cj].rearrange("p (i c) -> p i c", c=cj),
                                 w1ps[0:64, wcols],
                                 strided_half(x1s, 0, j, cj),
                                 start=True, stop=True)
                nc.tensor.matmul(ptb[:, 1, 0:4 * cj].rearrange("p (i c) -> p i c", c=cj),
                                 w1ps[64:128, wcols],
                                 strided_half(x1s, 64, j, cj),
                                 start=True, stop=True)
                src_ap = ptb[:, :, 0:4 * cj].rearrange("p b (i c) -> p b i c", c=cj)
                evict(strided(dst_tile, j, 0, 8, cj), src_ap,
                      b1ps[:, mo * M + j: mo * M + j + 1], True, 8 * cj)

            # ---- L2: h = relu(W01[a0].T @ h1 + b01[a0])  [K=256 -> 2 accum chunks]
            # k-outer so consecutive matmuls reuse the loaded weights;
            # w01 is packed module-major: chunk kc of module i at (2*i+kc)*O0
            def l2_unit(i):
                pt = ps128.tile([128, 1024], F32, tag="ps")
                outs2 = bank2(pt, a0_splits)
                for kc, src in enumerate((h1a, h1b)):
                    for s, (pos, sz) in enumerate(a0_splits):
                        nc.tensor.matmul(
                            outs2[s],
                            w01s[:, (2 * i + kc) * O0: (2 * i + kc + 1) * O0],
                            src[:, i * S + pos: i * S + pos + sz],
                            start=(kc == 0), stop=(kc == 1))
                sz0 = a0_splits[0][1]
                src_ap = pt[:].rearrange("p (b c) -> p b c", b=2)[:, :, 0:sz0]
                dst_ap = hs[:, i * S: i * S + S].rearrange("p (b c) -> p b c", b=2)
                evict(dst_ap, src_ap, b01s[:, i: i + 1], True, S)

            # Interleave L3 and L2 units: their dependency chains are
            # independent (L3: x1/w1p, L2: h1/w01), so stalls in one chain
            # are covered by ready work from the other.  L3 leads by 4 units
            # to cover w01's later DMA arrival.
            units = [("l3", j, mo) for mo in range(2) for j in range(M)]
            l2q = [("l2", i, None) for i in range(M)]
            merged = units[:4]
            rest = units[4:]
            k2 = 0
            for t, u in enumerate(rest):
                merged.append(u)
                if (t * M) // len(rest) != ((t + 1) * M) // len(rest):
                    if k2 < M:
                        merged.append(l2q[k2])
                        k2 += 1
            merged.extend(l2q[k2:])
            for kind, aa, bb in merged:
                if kind == "l3":
                    l3_unit(aa, bb)
                else:
                    l2_unit(aa)

            # ---- L4: g = relu(W1a[a1].T @ concat(h, g1) + b1a[a1]) [K=384 -> 3 chunks]
            # g reuses h1a's slot (h1 is dead after L2); k-outer for LDW reuse.
            # The g1 chunks accumulate first so L4's matmuls can start before
            # the last L2 (hs) evictions have landed.
            gs = acts.tile([128, ncol], DT, tag="h1a")
            outs_sb = acts.tile([128, 2 * L5W], DT, tag="h1b")

            def l4_unit(j):
                cj = Cj[j]
                pt = ps128.tile([128, 1024], F32, tag="ps")
                ptb = pt[:].rearrange("p (b c) -> p b c", b=2)
                for kc, src in ((1, g1a), (2, g1b), (0, hs)):
                    for s, (b0, nb) in enumerate(blk_splits):
                        ptv = ptb[:, s, 0:nb * cj].rearrange("p (i c) -> p i c", c=cj)
                        nc.tensor.matmul(
                            ptv,
                            w1as[:, (kc * M + j) * O0: (kc * M + j + 1) * O0],
                            strided(src, j, b0, nb, cj),
                            start=(kc == 1), stop=(kc == 0))
                src_ap = ptb[:, :, 0:4 * cj].rearrange("p b (i c) -> p b i c", c=cj)
                evict(strided(gs, j, 0, 8, cj), src_ap,
                      b1as[:, j: j + 1], True, 8 * cj)

            # ---- L5: out = W1o[a1].T @ g + b1o[a1]  (identity, out=32)
            # 4 a1-modules packed per PSUM tile via column tiling: module j of
            # group G lands on psum partitions 32*(j-4G)..+32.  i-blocks are
            # written at a uniform stride Wblk so one flat evict covers the
            # whole group; the output DMA is then a contiguous [128, L5W] blob.
            # Each group is emitted right after its 4 L4 units so the L5 tail
            # overlaps the other group's L4 work.
            def l5_group(G):
                pt = ps128.tile([128, 1024], F32, tag="ps")
                for jb in range(4):
                    j = 4 * G + jb
                    cj = Cj[j]
                    band = pt[32 * jb:32 * jb + 32, :]
                    for s, (b0, nb) in enumerate(blk_splits):
                        ptv = band.rearrange("p (b r) -> p b r", b=2)[:, s, 0:nb * Wblk]
                        ptv = ptv.rearrange("p (i c) -> p i c", c=Wblk)[:, :, 0:cj]
                        nc.tensor.matmul(
                            ptv,
                            w1os[:, j * O1: (j + 1) * O1],
                            strided(gs, j, b0, nb, cj),
                            start=True, stop=True,
                            tile_position=(0, 32 * jb))
                # split the eviction across both engines (one bank each) so
                # the final-group latency is halved on the critical tail
                # per-bank evict + per-bank output DMA so the first half's
                # transfer starts while the second half is still evicting
                for s in range(2):
                    src_ap = pt[:, s * 512:s * 512 + 4 * Wblk]
                    dst_ap = outs_sb[:, G * L5W + s * 4 * Wblk:
                                     G * L5W + (s + 1) * 4 * Wblk]
                    if s == 0:
                        nc.vector.tensor_scalar_add(dst_ap, src_ap, b1o5[:, G:G + 1])
                    else:
                        nc.scalar.activation(dst_ap, src_ap, Identity,
                                             bias=b1o5[:, G:G + 1])
                    nc.sync.dma_start(
                        outT[:, G * L5W + s * 4 * Wblk:G * L5W + (s + 1) * 4 * Wblk],
                        outs_sb[:, G * L5W + s * 4 * Wblk:G * L5W + (s + 1) * 4 * Wblk])

            for j in range(4):
                l4_unit(j)
            l5_group(0)
            for j in range(4, M):
                l4_unit(j)
            l5_group(1)

            # ---- gated tail dummies: the first one READS the final L5
            # eviction's SBUF output, so it fires only once the last real
            # work lands; the rest chain after it.  They run entirely in the
            # PE-idle shadow of the output-DMA drain (no body extension) and
            # keep the full duty-cycle grant alive into the epilogue, where
            # the Tensor sequencer's ~50 per-sem clears run at the PE clock
            # (57ns hot vs 115ns throttled).
            if TAIL_MMS:
                last = outs_sb[:, 2 * L5W - 128:2 * L5W]
                pw = ps128.tile([128, 1024], F32, tag="ps")
                nc.tensor.matmul(pw[:, 0:128], wu[:, 0:128], last,
                                 start=True, stop=True)
                for _ in range(TAIL_MMS - 1):
                    pw = ps128.tile([128, 1024], F32, tag="ps")
                    nc.tensor.matmul(pw[:, 0:128], wu[:, 0:128], last,
                                     start=True, stop=True)

    nc.compile()
    return nc


def _pack_weights(inputs):
    """Pack per-module weight stacks into SBUF-image layouts (contraction dim on
    partitions, [K<=128, chunks*modules*out] on the free axis)."""
    import ml_dtypes
    wdt = ml_dtypes.bfloat16
    f = lambda a: np.ascontiguousarray(a.astype(wdt))
    W00 = inputs["W00"]; W01 = inputs["W01"]; W1p = inputs["W1p"]
    W1a = inputs["W1a"]; W1o = inputs["W1o"]
    w00lo = W00[:M // 2].transpose(1, 0, 2).reshape(D0, M * H // 2)
    w00hi = W00[M // 2:].transpose(1, 0, 2).reshape(D0, M * H // 2)
    w1pp = W1p.transpose(1, 0, 2).reshape(D1, M * H)
    w01 = W01.reshape(M, 2, 128, O0).transpose(2, 0, 1, 3).reshape(128, 2 * M * O0)
    w1a = W1a.reshape(M, 3, 128, O0).transpose(2, 1, 0, 3).reshape(128, 3 * M * O0)
    w1o = W1o.transpose(1, 0, 2).reshape(128, M * O1)
    # biases: b00[16] | b01[8] | b1p[16] | b1a[8] | b1o5[2], fp32 [128, 50]
    b00 = inputs["b00"].reshape(M, 2, 128).transpose(2, 1, 0).reshape(128, 2 * M)
    b1p = inputs["b1p"].reshape(M, 2, 128).transpose(2, 1, 0).reshape(128, 2 * M)
    b01 = np.broadcast_to(inputs["b01"].T, (128, M))
    b1a = np.broadcast_to(inputs["b1a"].T, (128, M))
    # L5 col-tiled bias: partition 32*jb + ch of group G = b1o[4G + jb, ch]
    b1o5 = inputs["b1o"].reshape(2, 4 * O1).T               # [128, 2]
    biases = np.concatenate([b00, b01, b1p, b1a, b1o5], axis=1).astype(np.float32)
    return {
        "w00": f(np.concatenate([w00lo, w00hi], axis=0)),
        "w1p": f(np.concatenate([w1pp, w1pp], axis=0)),
        "wlt": f(np.concatenate([w01, w1a, w1o], axis=1)),
        "bt": np.ascontiguousarray(biases),
    }


def _route(input_val):
    """Assign each sample to a (core, column) in the blocked layout."""
    a0 = np.argmax(input_val[:, D0 + D1: D0 + D1 + M], axis=1)
    a1 = np.argmax(input_val[:, D0 + D1 + M: D0 + D1 + 2 * M], axis=1)
    B = input_val.shape[0]
    nij = np.zeros((M, M), dtype=np.int64)
    np.add.at(nij, (a0, a1), 1)
    Cj = np.maximum(-(-nij.max(axis=0) // NCORES), 64)
    off = np.concatenate([[0], np.cumsum(Cj)[:-1]]).astype(np.int64)
    S = int(Cj.sum())
    ncol = M * S

    pairkey = a0 * M + a1
    order = np.argsort(pairkey, kind="stable")
    counts = np.bincount(pairkey, minlength=M * M)
    group_start = np.concatenate([[0], np.cumsum(counts)[:-1]])
    rank_sorted = np.arange(B) - np.repeat(group_start, counts)
    rank = np.empty(B, dtype=np.int64)
    rank[order] = rank_sorted
    core = rank % NCORES
    slot = rank // NCORES
    assert np.all(slot < Cj[a1]), "capacity overflow"
    col = a0 * S + off[a1] + slot
    return core, col, a0, a1, slot, S, [int(c) for c in Cj], [int(o) for o in off], ncol


def kernel(**inputs):
    import os
    import ml_dtypes
    from concourse.bass_utils import run_bass_kernel_spmd

    input_val = np.asarray(inputs["input_val"], dtype=np.float32)
    B = input_val.shape[0]

    core, col, a0, a1, slot, S, Cj, off, ncol = _route(input_val)

    xdt = ml_dtypes.bfloat16
    feat0 = input_val[:, :D0]
    feat1 = input_val[:, D0:D0 + D1]
    # split layout: module blocks i<4 (by a0) on partitions 0-63, i>=4 on 64-127
    half = ncol // 2
    X0T = np.zeros((NCORES, 128, half), dtype=xdt)
    X1T = np.zeros((NCORES, 128, half), dtype=xdt)
    hi = col >= half
    prow = np.where(hi, 64, 0)
    pcol = np.where(hi, col - half, col)
    for r in (0, 64):
        m = prow == r
        X0T[core[m], r:r + 64, pcol[m]] = feat0[m].astype(xdt)
        X1T[core[m], r:r + 64, pcol[m]] = feat1[m].astype(xdt)

    wmap = _pack_weights({k: np.asarray(v, dtype=np.float32) for k, v in inputs.items()
                          if k != "input_val"})

    nc = _build_bass(S, Cj, off, ncol)

    in_maps = [dict(wmap, x0t=np.ascontiguousarray(X0T[c]),
                    x1t=np.ascontiguousarray(X1T[c])) for c in range(NCORES)]
    res = run_bass_kernel_spmd(nc, in_maps, core_ids=list(range(NCORES)),
                               tmpdir=os.environ.get("BASS_TMPDIR"))
    global _LAST_RESULTS
    _LAST_RESULTS = res

    # unshard: OUT[core, 32*(a1%4)+ch, (a1>=4)*L5W + (a0>=4)*4*Wblk + (a0%4)*Wblk + slot]
    Wblk = max(Cj)
    L5W = 8 * Wblk
    OUT = np.stack([np.asarray(r["outT"], dtype=np.float32) for r in res.results])
    OUT = OUT.reshape(NCORES, 4, O1, 2 * L5W)
    colnew = (a1 // 4) * L5W + (a0 // 4) * 4 * Wblk + (a0 % 4) * Wblk + slot
    out = OUT[core, a1 % 4, :, colnew]
    return np.ascontiguousarray(out).astype(np.float32)

